# revision 1
# baseline (speedup 1.0000x reference)
"""DVGCL (GNN message passing + contrastive losses) on 8 Trainium2 cores.

Sharding: node dim N split 8 ways by destination; each shard degree-sorted and
laid out cyclically (pos j -> partition j%128, col j//128); the permutation is
folded into every index array on the host. The symmetric normalization
separates (g = d_inv[h] d_inv[t]), so propagation gathers the pre-scaled table
y = d_inv * cur unweighted and rescales shard outputs. Per 128-dest tile,
slot-wise indirect_dma_start with compute_op=add accumulates gathered rows in
SBUF via the DMA CCE units; sentinel indices are skipped via bounds_check.
Layers are separated by AllGather of the 8 shard updates (double-buffered
table). Losses are computed as per-core partials (batch rotated per core so
its 1/8 slice is always at columns [0, BSHC)) and combined on the host.

Walrus codegen accepts at most ONE sync wait per instruction, so
split_multi_waits hoists extras onto same-engine NoOps after Tile scheduling.
"""
import math
import numpy as np

import concourse.bass as bass
import concourse.mybir as mybir
import concourse.tile as tile
from concourse.bass_utils import run_bass_kernel_spmd
from concourse.masks import make_identity

F32 = mybir.dt.float32
I32 = mybir.dt.int32
AX = mybir.AxisListType
ALU = mybir.AluOpType
ACTF = mybir.ActivationFunctionType

SENT = 1 << 20


def default_cfg():
    return dict(
        N_USERS=50000, N_ITEMS=100000, D=64, N_LAYERS=3, N_INTENTS=128,
        T_SIZE=32, TEMP=0.2, KL_REG=0.01, EMB_REG=1e-5, INT_REG=1e-5,
        SSL_REG=0.1, B=4096, NC=8,
    )


def derive(cfg):
    c = dict(cfg)
    c["N"] = c["N_USERS"] + c["N_ITEMS"]
    assert c["N"] % c["NC"] == 0
    c["SHARD"] = c["N"] // c["NC"]
    c["PC"] = math.ceil(c["SHARD"] / 128)
    c["SPAD"] = 128 * c["PC"]
    c["TROWS"] = c["NC"] * c["SPAD"]
    assert c["B"] % 128 == 0 and (c["B"] // c["NC"]) % 128 == 0
    c["BCOLS"] = c["B"] // 128
    c["BSH"] = c["B"] // c["NC"]
    c["BSHC"] = c["BSH"] // 128
    return c


# --------------------------------------------------------------------------
# wait splitting post-pass (walrus: max 1 sync wait per instruction)
# --------------------------------------------------------------------------

def split_multi_waits(nc, max_waits=1):
    n = 0
    for f in nc.m.functions:
        for b in f.blocks:
            insts = b.instructions
            items = list(insts)
            out = []
            for i in items:
                si = i.sync_info
                w = list(si.on_wait) if si and si.on_wait else []
                if len(w) > max_waits:
                    for x in w[:-max_waits]:
                        n += 1
                        out.append(mybir.InstNoOp(
                            name=f"waitsplit-{n}",
                            sync_info=mybir.SyncInfo(on_wait=[x], on_update=[]),
                            engine=i.engine, bass_nofuse=True))
                    si.on_wait = w[-max_waits:]
                out.append(i)
            insts.clear()
            insts.extend(out)
    return n


# --------------------------------------------------------------------------
# host prep
# --------------------------------------------------------------------------

def host_prep(inputs, c):
    N, NC, SHARD, SPAD, PC, D = (c["N"], c["NC"], c["SHARD"], c["SPAD"],
                                 c["PC"], c["D"])
    h = np.asarray(inputs["h_list"]).astype(np.int64)
    t = np.asarray(inputs["t_list"]).astype(np.int64)

    deg = np.bincount(h, minlength=N).astype(np.int64)
    with np.errstate(divide="ignore"):
        d_inv = (deg.astype(np.float64) ** -0.5).astype(np.float32)

    perm_row = np.empty(N, dtype=np.int64)
    inv_order = []
    for k in range(NC):
        lo = k * SHARD
        order = np.argsort(deg[lo:lo + SHARD], kind="stable")
        perm_row[lo + order] = k * SPAD + np.arange(SHARD)
        inv_order.append(lo + order)

    dest_pos = perm_row[h]
    eorder = np.argsort(dest_pos, kind="stable")
    dpos_s = dest_pos[eorder]
    src_s = perm_row[t[eorder]]

    ego = np.concatenate([
        np.asarray(inputs["user_emb"], dtype=np.float32),
        np.asarray(inputs["item_emb"], dtype=np.float32),
    ], axis=0)

    # per-core per-tile slot columns
    core_cols = []       # list of dict[(tau, s)] -> int32[128]
    core_smax = []       # per core: [PC] slot counts
    core_zero = []       # per core: [PC] tile has a zero-degree dest
    for k in range(NC):
        base = k * SPAD
        lo_i = np.searchsorted(dpos_s, base)
        hi_i = np.searchsorted(dpos_s, base + SHARD)
        dj = dpos_s[lo_i:hi_i] - base
        sj = src_s[lo_i:hi_i]
        degl = np.zeros(SPAD, dtype=np.int64)
        np.add.at(degl, dj, 1)
        starts = np.zeros(SPAD + 1, dtype=np.int64)
        np.cumsum(degl, out=starts[1:])
        cols = {}
        smax = np.zeros(PC, dtype=np.int64)
        zero = np.zeros(PC, dtype=bool)
        for tau in range(PC):
            jlo = tau * 128
            dtile = degl[jlo:jlo + 128]
            smax[tau] = int(dtile.max())
            zero[tau] = bool((dtile == 0).any())
            for s in range(smax[tau]):
                col = np.full(128, SENT, dtype=np.int64)
                sel = dtile > s
                col[sel] = sj[starts[jlo:jlo + 128][sel] + s]
                cols[(tau, s)] = col.astype(np.int32)
        core_cols.append(cols)
        core_smax.append(smax)
        core_zero.append(zero)

    # SPMD union plan
    smax_u = np.max(np.stack(core_smax), axis=0)
    zero_u = np.any(np.stack(core_zero), axis=0)
    uplan = []
    for tau in range(PC):
        s = int(smax_u[tau])
        if s == 0:
            uplan.append(dict(tau=tau, memset=True, ops=[]))
        elif zero_u[tau]:
            uplan.append(dict(tau=tau, memset=True, ops=["add"] * s))
        else:
            uplan.append(dict(tau=tau, memset=False,
                              ops=["bypass"] + ["add"] * (s - 1)))
    NI = max(1, int(smax_u.sum()))

    per_core = []
    for k in range(NC):
        out_cols = []
        for e in uplan:
            for s in range(len(e["ops"])):
                col = core_cols[k].get((e["tau"], s))
                if col is None:
                    col = np.full(128, SENT, dtype=np.int32)
                out_cols.append(col)
        idx_spmm = (np.stack(out_cols, axis=1) if out_cols
                    else np.zeros((128, 1), np.int32))

        def cyc(vec):
            return vec.reshape(PC, 128).T.copy()

        dloc = np.zeros(SPAD, dtype=np.float32)
        dloc[:SHARD] = d_inv[inv_order[k]]
        mask = np.zeros(SPAD, dtype=np.float32)
        mask[:SHARD] = 1.0
        egp = np.zeros((SPAD, D), dtype=np.float32)
        egp[:SHARD] = ego[inv_order[k]]
        per_core.append(dict(
            idx_spmm=idx_spmm, dinv=cyc(dloc), dinv2=cyc(dloc * dloc),
            kmask=cyc(mask), ego_perm=egp))

    return per_core, ego, perm_row, uplan, NI


# --------------------------------------------------------------------------
# device program
# --------------------------------------------------------------------------

def build_bass(c):
    NC, D, PC, SPAD, TROWS = c["NC"], c["D"], c["PC"], c["SPAD"], c["TROWS"]
    BC, BSHC, NI = c["BCOLS"], c["BSHC"], c["NI"]
    NINT, TS, NL = c["N_INTENTS"], c["T_SIZE"], c["N_LAYERS"]
    TEMP = c["TEMP"]
    uplan = c["UPLAN"]
    NB = 2 * BC + 8 * BSHC

    nc = bass.Bass(num_devices=NC)

    ego_perm = nc.dram_tensor("ego_perm", [SPAD, D], F32, kind="ExternalInput")
    idx_spmm = nc.dram_tensor("idx_spmm", [128, NI], I32, kind="ExternalInput")
    dinv_in = nc.dram_tensor("dinv", [128, PC], F32, kind="ExternalInput")
    dinv2_in = nc.dram_tensor("dinv2", [128, PC], F32, kind="ExternalInput")
    kmask_in = nc.dram_tensor("kmask", [128, PC], F32, kind="ExternalInput")
    idx_b_in = nc.dram_tensor("idx_b", [128, NB], I32, kind="ExternalInput")
    eps_in = nc.dram_tensor("eps", [c["N"], D], F32, kind="ExternalInput")
    ego_full = nc.dram_tensor("ego_full", [c["N"], D], F32,
                              kind="ExternalInput")
    ui_in = nc.dram_tensor("user_intent", [D, NINT], F32, kind="ExternalInput")
    ii_in = nc.dram_tensor("item_intent", [D, NINT], F32, kind="ExternalInput")
    lw_in = nc.dram_tensor("lin_w", [D, TS], F32, kind="ExternalInput")
    lb_in = nc.dram_tensor("lin_b_rep", [128, D], F32, kind="ExternalInput")

    partials = nc.dram_tensor("partials", [1, 16], F32, kind="ExternalOutput")

    BF16 = mybir.dt.bfloat16
    yA = nc.dram_tensor("yA", [TROWS, D], BF16, addr_space="Shared")
    yB = nc.dram_tensor("yB", [TROWS, D], BF16, addr_space="Shared")
    all_emb = nc.dram_tensor("all_emb", [TROWS, D], F32, addr_space="Shared")
    shard_buf = nc.dram_tensor("shard_buf", [SPAD, D], F32)
    shard_bf = nc.dram_tensor("shard_bf", [SPAD, D], BF16)

    groups = [list(range(NC))]

    # batch idx column offsets
    OFF_IU, OFF_IP = 0, BC
    OFF_US = 2 * BC                 # ua shard (perm)
    OFF_PS = OFF_US + BSHC          # ia pos shard (perm)
    OFF_NS = OFF_US + 2 * BSHC      # ia neg shard (perm)
    OFF_EU = OFF_US + 3 * BSHC      # eps users shard (orig)
    OFF_EP = OFF_US + 4 * BSHC      # eps pos shard (orig)
    OFF_GU = OFF_US + 5 * BSHC      # ego users shard (orig)
    OFF_GP = OFF_US + 6 * BSHC
    OFF_GN = OFF_US + 7 * BSHC

    with tile.TileContext(nc) as tc:
        with tc.tile_pool(name="const", bufs=1) as cp, \
             tc.tile_pool(name="work", bufs=2) as wp, \
             tc.tile_pool(name="spmm", bufs=c["PC"] + 8) as sp, \
             tc.tile_pool(name="curp", bufs=8) as curp, \
             tc.tile_pool(name="psum", bufs=1, space="PSUM") as pp:

            ident = cp.tile([128, 128], F32)
            make_identity(nc, ident[:])
            bc_reg = nc.alloc_register(mybir.EngineType.Pool, "bcreg")
            nc.gpsimd.reg_mov(bc_reg, TROWS - 1)
            ones_col = cp.tile([128, 1], F32)
            nc.vector.memset(ones_col[:], 1.0)

            def load(shape, dt, src, name):
                t_ = cp.tile(shape, dt, tag=name)
                nc.sync.dma_start(out=t_[:], in_=src)
                return t_

            idxs = load([128, NI], I32, idx_spmm[:], "idxs")
            dinv = load([128, PC], F32, dinv_in[:], "dinv")
            dinv2 = load([128, PC], F32, dinv2_in[:], "dinv2")
            kmask = load([128, PC], F32, kmask_in[:], "kmask")
            idxb = load([128, NB], I32, idx_b_in[:], "idxb")
            lb_rep = load([128, D], F32, lb_in[:], "lb")
            ui_sb = load([D, NINT], F32, ui_in[:], "ui")
            ii_sb = load([D, NINT], F32, ii_in[:], "ii")
            lw_sb = load([D, TS], F32, lw_in[:], "lw")

            def transpose128(dst_ap, src_ap, P, Fr):
                # src [P, Fr] -> dst [Fr, P]
                ps = pp.tile([128, 128], F32, tag="tps")
                nc.tensor.transpose(out=ps[:Fr, :P], in_=src_ap,
                                    identity=ident[:P, :P])
                nc.vector.tensor_copy(dst_ap, ps[:Fr, :P])

            uiT = cp.tile([NINT, D], F32)
            transpose128(uiT[:], ui_sb[:], D, NINT)
            iiT = cp.tile([NINT, D], F32)
            transpose128(iiT[:], ii_sb[:], D, NINT)
            lwT = cp.tile([TS, D], F32)
            transpose128(lwT[:], lw_sb[:], D, TS)

            acc = cp.tile([128, PC * D], F32)
            nc.sync.dma_start(
                out=acc[:].rearrange("p (c d) -> p c d", d=D),
                in_=ego_perm[:].rearrange("(c p) d -> p c d", p=128))

            def dcol(tbl, tau):
                return tbl[:, tau:tau + 1].to_broadcast([128, D])

            def sbcol(tau):
                return shard_bf[:].rearrange(
                    "(c p) d -> p c d", p=128)[:, tau, :]

            # y0 = d_inv * ego, per column (cast to bf16 for the y table)
            for tau in range(PC):
                yt = sp.tile([128, D], F32, tag="out_t")
                nc.vector.tensor_tensor(
                    out=yt[:], in0=acc[:, tau * D:(tau + 1) * D],
                    in1=dcol(dinv, tau), op=ALU.mult)
                yb = curp.tile([128, D], BF16, tag="ybf")
                nc.vector.tensor_copy(yb[:], yt[:])
                nc.sync.dma_start(out=sbcol(tau), in_=yb[:])
            nc.gpsimd.collective_compute(
                "AllGather", ALU.bypass, replica_groups=groups,
                ins=[shard_bf[:]], outs=[yA[:]])

            smax_all = max((len(e["ops"]) for e in uplan), default=0)
            # column index of instr (tau, s) inside idx_spmm
            colof = {}
            _ic = 0
            for e in uplan:
                for s in range(len(e["ops"])):
                    colof[(e["tau"], s)] = _ic
                    _ic += 1

            ybufs = [yA, yB]
            for layer in range(NL):
                y_in = ybufs[layer % 2]
                y_out = ybufs[(layer + 1) % 2]
                last = (layer == NL - 1)
                outs = {}
                for e in uplan:
                    out_t = sp.tile([128, D], F32, tag="out_t")
                    outs[e["tau"]] = out_t
                    if e["memset"]:
                        nc.vector.memset(out_t[:], 0.0)
                # slot-major round-robin: chain neighbors are ~PC apart in
                # the Pool stream, so DMA completions overlap fully
                for s in range(smax_all):
                    for e in uplan:
                        if s >= len(e["ops"]):
                            continue
                        op = e["ops"][s]
                        kw = {}
                        if op != "bypass":
                            kw = dict(bounds_check=bc_reg,
                                      oob_is_err=False, compute_op=ALU.add)
                        icol = colof[(e["tau"], s)]
                        nc.gpsimd.indirect_dma_start(
                            out=outs[e["tau"]][:], out_offset=None,
                            in_=y_in[:],
                            in_offset=bass.IndirectOffsetOnAxis(
                                ap=idxs[:, icol:icol + 1], axis=0),
                            **kw)
                for e in uplan:
                    tau = e["tau"]
                    out_t = outs[tau]
                    aslice = acc[:, tau * D:(tau + 1) * D]
                    cur = curp.tile([128, D], F32, tag="cur")
                    nc.vector.tensor_tensor(out=cur[:], in0=out_t[:],
                                            in1=dcol(dinv, tau), op=ALU.mult)
                    nc.vector.tensor_add(out=aslice, in0=aslice, in1=cur[:])
                    if not last:
                        # in-place: out_t becomes y_next column
                        nc.vector.tensor_tensor(
                            out=out_t[:], in0=out_t[:], in1=dcol(dinv2, tau),
                            op=ALU.mult)
                        yb = curp.tile([128, D], BF16, tag="ybf")
                        nc.vector.tensor_copy(yb[:], out_t[:])
                        nc.sync.dma_start(out=sbcol(tau), in_=yb[:])
                if not last:
                    nc.gpsimd.collective_compute(
                        "AllGather", ALU.bypass, replica_groups=groups,
                        ins=[shard_bf[:]], outs=[y_out[:]])

            nc.sync.dma_start(
                out=shard_buf[:].rearrange("(c p) d -> p c d", p=128),
                in_=acc[:].rearrange("p (c d) -> p c d", d=D))
            nc.gpsimd.collective_compute(
                "AllGather", ALU.bypass, replica_groups=groups,
                ins=[shard_buf[:]], outs=[all_emb[:]])

            # ---------------- downstream ----------------

            def gather1(src, col, dst_ap):
                nc.gpsimd.indirect_dma_start(
                    out=dst_ap, out_offset=None, in_=src[:],
                    in_offset=bass.IndirectOffsetOnAxis(
                        ap=idxb[:, col:col + 1], axis=0))

            def gather_set(src, off, ncols, tag, pool=cp):
                tl = pool.tile([128, ncols * D], F32, tag=tag)
                for q in range(ncols):
                    gather1(src, off + q, tl[:, q * D:(q + 1) * D])
                return tl

            ua_sh = gather_set(all_emb, OFF_US, BSHC, "ua_sh")
            iap_sh = gather_set(all_emb, OFF_PS, BSHC, "iap_sh")
            ian_sh = gather_set(all_emb, OFF_NS, BSHC, "ian_sh")
            eps_u = gather_set(eps_in, OFF_EU, BSHC, "eps_u")
            eps_p = gather_set(eps_in, OFF_EP, BSHC, "eps_p")
            ego_u = gather_set(ego_full, OFF_GU, BSHC, "ego_u")
            ego_p = gather_set(ego_full, OFF_GP, BSHC, "ego_p")
            ego_n = gather_set(ego_full, OFF_GN, BSHC, "ego_n")

            def normalize_rows(x_ap, ncols):
                for q in range(ncols):
                    sl = x_ap[:, q * D:(q + 1) * D]
                    sq = wp.tile([128, D], F32, tag="sqj")
                    ss = wp.tile([128, 1], F32, tag="ssj")
                    nc.scalar.activation(sq[:], sl, ACTF.Square,
                                         accum_out=ss[:])
                    ls = wp.tile([128, 1], F32, tag="rsj")
                    nc.scalar.activation(ls[:], ss[:], ACTF.Ln)
                    rn = wp.tile([128, 1], F32, tag="rnj")
                    nc.scalar.activation(rn[:], ls[:], ACTF.Exp, scale=-0.5)
                    nc.vector.tensor_scalar_mul(sl, sl, rn[:])

            def intent_pipe(gsrc_off, w_sb, wT_sb, tag):
                """Full-batch intent; returns (shard normalized [128,BSHC*D],
                e2T [D, B])."""
                sh_n = cp.tile([128, BSHC * D], F32, tag=f"in_{tag}")
                e2T = cp.tile([D, BC * 128], F32, tag=f"iT_{tag}")
                for q in range(BC):
                    tl = wp.tile([128, D], F32, tag="itl")
                    gather1(all_emb, gsrc_off + q, tl[:])
                    tT = wp.tile([D, 128], F32, tag="tT")
                    transpose128(tT[:], tl[:], 128, D)
                    zp = pp.tile([128, NINT], F32, tag="zp")
                    nc.tensor.matmul(out=zp[:], lhsT=tT[:], rhs=w_sb[:])
                    z = wp.tile([128, NINT], F32, tag="z")
                    nc.vector.tensor_copy(z[:], zp[:])
                    mx = wp.tile([128, 1], F32, tag="mx")
                    nc.vector.tensor_reduce(out=mx[:], in_=z[:], axis=AX.X,
                                            op=ALU.max)
                    nmx = wp.tile([128, 1], F32, tag="nmx")
                    nc.scalar.mul(nmx[:], mx[:], -1.0)
                    ex = wp.tile([128, NINT], F32, tag="ex")
                    se = wp.tile([128, 1], F32, tag="se")
                    nc.scalar.activation(ex[:], z[:], ACTF.Exp, bias=nmx[:],
                                         accum_out=se[:])
                    rse = wp.tile([128, 1], F32, tag="rse")
                    nc.vector.reciprocal(rse[:], se[:])
                    nc.vector.tensor_scalar_mul(ex[:], ex[:], rse[:])
                    exT = wp.tile([NINT, 128], F32, tag="exT")
                    transpose128(exT[:], ex[:], 128, NINT)
                    op_ = pp.tile([128, D], F32, tag="op")
                    nc.tensor.matmul(out=op_[:], lhsT=exT[:], rhs=wT_sb[:])
                    onrm = wp.tile([128, D], F32, tag="onrm")
                    nc.vector.tensor_copy(onrm[:], op_[:])
                    normalize_rows(onrm[:], 1)
                    if q < BSHC:
                        nc.vector.tensor_copy(
                            sh_n[:, q * D:(q + 1) * D], onrm[:])
                    transpose128(e2T[:, q * 128:(q + 1) * 128], onrm[:],
                                 128, D)
                return sh_n, e2T

            u_i_n, u_i_T = intent_pipe(OFF_IU, ui_sb, uiT, "u")
            i_i_n, i_i_T = intent_pipe(OFF_IP, ii_sb, iiT, "i")

            def gen_pipe(mean_tl, eps_tl, tag):
                gen_n = cp.tile([128, BSHC * D], F32, tag=f"gen_{tag}")
                genT = cp.tile([D, BSHC * 128], F32, tag=f"genT_{tag}")
                for q in range(BSHC):
                    msl = mean_tl[:, q * D:(q + 1) * D]
                    sp_t = wp.tile([128, TS], F32, tag="sp_t")
                    nc.scalar.activation(sp_t[:], msl[:, :TS], ACTF.Exp)
                    nc.vector.tensor_scalar_add(sp_t[:], sp_t[:], 1.0)
                    nc.scalar.activation(sp_t[:], sp_t[:], ACTF.Ln)
                    spT = wp.tile([TS, 128], F32, tag="spT")
                    transpose128(spT[:], sp_t[:], 128, TS)
                    stp = pp.tile([128, D], F32, tag="stp")
                    nc.tensor.matmul(out=stp[:], lhsT=spT[:], rhs=lwT[:])
                    std = wp.tile([128, D], F32, tag="std")
                    nc.vector.tensor_add(out=std[:], in0=stp[:],
                                         in1=lb_rep[:])
                    nc.vector.tensor_scalar_add(std[:], std[:], 1e-8)
                    g = gen_n[:, q * D:(q + 1) * D]
                    nc.vector.tensor_tensor(
                        out=g, in0=eps_tl[:, q * D:(q + 1) * D], in1=std[:],
                        op=ALU.mult)
                    nc.vector.tensor_add(out=g, in0=g, in1=msl)
                normalize_rows(gen_n[:], BSHC)
                for q in range(BSHC):
                    transpose128(genT[:, q * 128:(q + 1) * 128],
                                 gen_n[:, q * D:(q + 1) * D], 128, D)
                return gen_n, genT

            u_gen_n, u_gen_T = gen_pipe(ua_sh, eps_u, "gu")
            i_gen_n, i_gen_T = gen_pipe(iap_sh, eps_p, "gi")

            psb = cp.tile([1, 16], F32)
            nc.vector.memset(psb[:], 0.0)

            def part_sum(vec_ap, slot, P=128):
                ps = pp.tile([1, 1], F32, tag="pscal")
                nc.tensor.matmul(out=ps[:], lhsT=vec_ap, rhs=ones_col[:P, :])
                nc.vector.tensor_add(out=psb[:, slot:slot + 1],
                                     in0=psb[:, slot:slot + 1], in1=ps[:])

            def infonce(e1_n, e1_T, e2sh_n, e2_T, slot):
                lgs = wp.tile([128, BSHC], F32, tag="lgs")
                for q in range(BSHC):
                    prod = wp.tile([128, D], F32, tag="prod")
                    nc.vector.tensor_tensor(
                        out=prod[:], in0=e1_n[:, q * D:(q + 1) * D],
                        in1=e2sh_n[:, q * D:(q + 1) * D], op=ALU.mult)
                    pdot = wp.tile([128, 1], F32, tag="pdot")
                    nc.vector.tensor_reduce(out=pdot[:], in_=prod[:],
                                            axis=AX.X, op=ALU.add)
                    pex = wp.tile([128, 1], F32, tag="pex")
                    nc.scalar.activation(pex[:], pdot[:], ACTF.Exp,
                                         scale=1.0 / TEMP)
                    nss = wp.tile([128, BC], F32, tag="nss")
                    for ch in range(BC):
                        zp = pp.tile([128, 128], F32, tag="zneg")
                        nc.tensor.matmul(
                            out=zp[:], lhsT=e1_T[:, q * 128:(q + 1) * 128],
                            rhs=e2_T[:, ch * 128:(ch + 1) * 128])
                        ju = wp.tile([128, 128], F32, tag="ju")
                        nc.scalar.activation(
                            ju[:], zp[:], ACTF.Exp, scale=1.0 / TEMP,
                            accum_out=nss[:, ch:ch + 1])
                    nsum = wp.tile([128, 1], F32, tag="nsum")
                    nc.vector.tensor_reduce(out=nsum[:], in_=nss[:],
                                            axis=AX.X, op=ALU.add)
                    nc.vector.tensor_scalar_add(nsum[:], nsum[:], 1e-8)
                    rn = wp.tile([128, 1], F32, tag="rng")
                    nc.vector.reciprocal(rn[:], nsum[:])
                    qv = wp.tile([128, 1], F32, tag="qv")
                    nc.vector.tensor_tensor(out=qv[:], in0=pex[:], in1=rn[:],
                                            op=ALU.mult)
                    nc.vector.tensor_scalar_add(qv[:], qv[:], 1e-8)
                    nc.scalar.activation(lgs[:, q:q + 1], qv[:], ACTF.Ln)
                tot = wp.tile([128, 1], F32, tag="lgt")
                nc.vector.tensor_reduce(out=tot[:], in_=lgs[:], axis=AX.X,
                                        op=ALU.add)
                part_sum(tot[:], slot)

            infonce(u_gen_n, u_gen_T, u_i_n, u_i_T, 2)
            infonce(i_gen_n, i_gen_T, i_i_n, i_i_T, 3)

            # BPR
            dsc = wp.tile([128, BSHC], F32, tag="dsc")
            for q in range(BSHC):
                pr = wp.tile([128, D], F32, tag="bprp")
                nc.vector.tensor_tensor(
                    out=pr[:], in0=ua_sh[:, q * D:(q + 1) * D],
                    in1=iap_sh[:, q * D:(q + 1) * D], op=ALU.mult)
                ps_ = wp.tile([128, 1], F32, tag="bps")
                nc.vector.tensor_reduce(out=ps_[:], in_=pr[:], axis=AX.X,
                                        op=ALU.add)
                nr = wp.tile([128, D], F32, tag="bprn")
                nc.vector.tensor_tensor(
                    out=nr[:], in0=ua_sh[:, q * D:(q + 1) * D],
                    in1=ian_sh[:, q * D:(q + 1) * D], op=ALU.mult)
                ns_ = wp.tile([128, 1], F32, tag="bns")
                nc.vector.tensor_reduce(out=ns_[:], in_=nr[:], axis=AX.X,
                                        op=ALU.add)
                nc.vector.tensor_sub(out=dsc[:, q:q + 1], in0=ns_[:],
                                     in1=ps_[:])
            spl = wp.tile([128, BSHC], F32, tag="spl")
            nc.scalar.activation(spl[:], dsc[:], ACTF.Exp)
            nc.vector.tensor_scalar_add(spl[:], spl[:], 1.0)
            nc.scalar.activation(spl[:], spl[:], ACTF.Ln)
            bps = wp.tile([128, 1], F32, tag="bpst")
            nc.vector.tensor_reduce(out=bps[:], in_=spl[:], axis=AX.X,
                                    op=ALU.add)
            part_sum(bps[:], 0)

            # emb partial
            esums = wp.tile([128, 4], F32, tag="esums")
            for j, tl in enumerate([ego_u, ego_p, ego_n]):
                jk = wp.tile([128, BSHC * D], F32, tag="jnk")
                nc.scalar.activation(jk[:], tl[:], ACTF.Square,
                                     accum_out=esums[:, j:j + 1])
            nc.vector.memset(esums[:, 3:4], 0.0)
            etot = wp.tile([128, 1], F32, tag="etot")
            nc.vector.tensor_reduce(out=etot[:], in_=esums[:], axis=AX.X,
                                    op=ALU.add)
            part_sum(etot[:], 4)

            # int partial
            isums = wp.tile([D, 2], F32, tag="isums")
            for j, tl in enumerate([ui_sb, ii_sb]):
                jk2 = wp.tile([D, NINT], F32, tag="jnk2")
                nc.scalar.activation(jk2[:], tl[:], ACTF.Square,
                                     accum_out=isums[:, j:j + 1])
            itot = wp.tile([D, 1], F32, tag="itot")
            nc.vector.tensor_reduce(out=itot[:], in_=isums[:], axis=AX.X,
                                    op=ALU.add)
            part_sum(itot[:], 5, P=D)

            # KL over own shard (from acc)
            KW = 8
            klcols = cp.tile([128, PC], F32)
            for g in range(math.ceil(PC / KW)):
                w0 = g * KW
                W = min(KW, PC - w0)
                mean_g = acc[:, w0 * D:(w0 + W) * D]
                spg = wp.tile([128, KW * TS], F32, tag="spg")
                nc.scalar.activation(
                    spg[:, :W * TS].rearrange("p (c d) -> p c d", d=TS),
                    acc[:, w0 * D:].rearrange(
                        "p (c d) -> p c d", d=D)[:, 0:W, 0:TS],
                    ACTF.Exp)
                nc.vector.tensor_scalar_add(spg[:, :W * TS], spg[:, :W * TS],
                                            1.0)
                nc.scalar.activation(spg[:, :W * TS], spg[:, :W * TS],
                                     ACTF.Ln)
                stdg = wp.tile([128, KW * D], F32, tag="stdg")
                for w in range(W):
                    spT = wp.tile([TS, 128], F32, tag="spTk")
                    transpose128(spT[:], spg[:, w * TS:(w + 1) * TS], 128, TS)
                    stp = pp.tile([128, D], F32, tag="stp")
                    nc.tensor.matmul(out=stp[:], lhsT=spT[:], rhs=lwT[:])
                    sw = stdg[:, w * D:(w + 1) * D]
                    nc.vector.tensor_add(out=sw, in0=stp[:], in1=lb_rep[:])
                    nc.vector.tensor_scalar_add(sw, sw, 1e-8)
                m2 = wp.tile([128, KW * D], F32, tag="m2")
                nc.scalar.activation(m2[:, :W * D], mean_g, ACTF.Square)
                exg = wp.tile([128, KW * D], F32, tag="exg")
                nc.scalar.activation(exg[:, :W * D], stdg[:, :W * D],
                                     ACTF.Exp, scale=2.0)
                t1 = wp.tile([128, KW * D], F32, tag="t1")
                nc.scalar.activation(t1[:, :W * D], stdg[:, :W * D],
                                     ACTF.Copy, bias=0.0, scale=2.0)
                nc.vector.tensor_scalar_add(t1[:, :W * D], t1[:, :W * D], 1.0)
                nc.vector.tensor_sub(out=t1[:, :W * D], in0=t1[:, :W * D],
                                     in1=m2[:, :W * D])
                nc.vector.tensor_sub(out=t1[:, :W * D], in0=t1[:, :W * D],
                                     in1=exg[:, :W * D])
                nc.vector.tensor_reduce(
                    out=klcols[:, w0:w0 + W],
                    in_=t1[:, :W * D].rearrange("p (c d) -> p c d", d=D),
                    axis=AX.X, op=ALU.add)
            nc.vector.tensor_tensor(out=klcols[:], in0=klcols[:],
                                    in1=kmask[:], op=ALU.mult)
            ktot = wp.tile([128, 1], F32, tag="ktot")
            nc.vector.tensor_reduce(out=ktot[:], in_=klcols[:], axis=AX.X,
                                    op=ALU.add)
            part_sum(ktot[:], 1)

            nc.sync.dma_start(out=partials[:], in_=psb[:])

    return nc


# --------------------------------------------------------------------------
# entry
# --------------------------------------------------------------------------

def prepare(inputs, c):
    """Returns (nc, in_maps)."""
    NC = c["NC"]
    per_core, ego, perm_row, uplan, NI = host_prep(inputs, c)
    c["UPLAN"] = uplan
    c["NI"] = NI

    users0 = np.asarray(inputs["users"]).astype(np.int64)
    pos0 = np.asarray(inputs["pos_items"]).astype(np.int64)
    neg0 = np.asarray(inputs["neg_items"]).astype(np.int64)
    N_USERS, B, BSH = c["N_USERS"], c["B"], c["BSH"]

    def cycb(v):
        m = len(v) // 128
        return v.reshape(m, 128).T.astype(np.int32)

    eps_np = np.asarray(inputs["eps"], dtype=np.float32)
    ui_np = np.asarray(inputs["user_intent"], dtype=np.float32)
    ii_np = np.asarray(inputs["item_intent"], dtype=np.float32)
    lw_np = np.asarray(inputs["lin_w"], dtype=np.float32)
    lb_rep = np.tile(np.asarray(inputs["lin_b"],
                                dtype=np.float32)[None, :], (128, 1))

    in_maps = []
    for k in range(NC):
        rot = np.roll(np.arange(B), -k * BSH)
        users, pos, neg = users0[rot], pos0[rot], neg0[rot]
        sh = slice(0, BSH)
        idx_b = np.concatenate([
            cycb(perm_row[users]),                    # OFF_IU (full)
            cycb(perm_row[N_USERS + pos]),            # OFF_IP (full)
            cycb(perm_row[users[sh]]),                # OFF_US
            cycb(perm_row[N_USERS + pos[sh]]),        # OFF_PS
            cycb(perm_row[N_USERS + neg[sh]]),        # OFF_NS
            cycb(users[sh]),                          # OFF_EU
            cycb(N_USERS + pos[sh]),                  # OFF_EP
            cycb(users[sh]),                          # OFF_GU
            cycb(N_USERS + pos[sh]),                  # OFF_GP
            cycb(N_USERS + neg[sh]),                  # OFF_GN
        ], axis=1)
        pk = per_core[k]
        in_maps.append(dict(
            ego_perm=pk["ego_perm"], idx_spmm=pk["idx_spmm"],
            dinv=pk["dinv"], dinv2=pk["dinv2"], kmask=pk["kmask"],
            idx_b=idx_b, eps=eps_np, ego_full=ego, user_intent=ui_np,
            item_intent=ii_np, lin_w=lw_np, lin_b_rep=lb_rep))

    nc = build_bass(c)
    split_multi_waits(nc)
    return nc, in_maps


def combine(results, c):
    NC, B, N = c["NC"], c["B"], c["N"]
    P = np.stack([np.asarray(results[k]["partials"][0], dtype=np.float64)
                  for k in range(NC)])
    bpr = P[:, 0].sum() / B
    kl = c["KL_REG"] * (-0.5 * P[:, 1].sum()) / N
    gen_loss = np.float32(bpr + kl)
    cl_loss = np.float32(c["SSL_REG"] * (-(P[:, 2].sum()) - P[:, 3].sum()) / B)
    emb_loss = np.float32(c["EMB_REG"] * P[:, 4].sum())
    int_loss = np.float32(c["INT_REG"] * P[0, 5])
    return (gen_loss, cl_loss, emb_loss, int_loss)


def kernel(**inputs):
    c = derive(default_cfg())
    nc, in_maps = prepare(inputs, c)
    res = run_bass_kernel_spmd(nc, in_maps, list(range(c["NC"])))
    return combine(res.results, c)



# revision 23
# speedup vs baseline: 1.0150x; 1.0150x over previous
"""DVGCL (GNN message passing + contrastive losses) on 8 Trainium2 cores.

Sharding: node dim N split 8 ways by destination; each shard degree-sorted and
laid out cyclically (pos j -> partition j%128, col j//128); the permutation is
folded into every index array on the host. The symmetric normalization
separates (g = d_inv[h] d_inv[t]), so propagation gathers the pre-scaled table
y = d_inv * cur unweighted and rescales shard outputs. Per 128-dest tile,
slot-wise indirect_dma_start with compute_op=add accumulates gathered rows in
SBUF via the DMA CCE units; sentinel indices are skipped via bounds_check.
Layers are separated by AllGather of the 8 shard updates (double-buffered
table). Losses are computed as per-core partials (batch rotated per core so
its 1/8 slice is always at columns [0, BSHC)) and combined on the host.

Walrus codegen accepts at most ONE sync wait per instruction, so
split_multi_waits hoists extras onto same-engine NoOps after Tile scheduling.
"""
import math
import numpy as np

import concourse.bass as bass
import concourse.mybir as mybir
import concourse.tile as tile
from concourse.bass_utils import run_bass_kernel_spmd
from concourse.masks import make_identity

F32 = mybir.dt.float32
I32 = mybir.dt.int32
AX = mybir.AxisListType
ALU = mybir.AluOpType
ACTF = mybir.ActivationFunctionType

SENT = 1 << 20


def default_cfg():
    return dict(
        N_USERS=50000, N_ITEMS=100000, D=64, N_LAYERS=3, N_INTENTS=128,
        T_SIZE=32, TEMP=0.2, KL_REG=0.01, EMB_REG=1e-5, INT_REG=1e-5,
        SSL_REG=0.1, B=4096, NC=8,
    )


def derive(cfg):
    c = dict(cfg)
    c["N"] = c["N_USERS"] + c["N_ITEMS"]
    assert c["N"] % c["NC"] == 0
    c["SHARD"] = c["N"] // c["NC"]
    c["PC"] = math.ceil(c["SHARD"] / 128)
    c["SPAD"] = 128 * c["PC"]
    c["TROWS"] = c["NC"] * c["SPAD"]
    assert c["B"] % 128 == 0 and (c["B"] // c["NC"]) % 128 == 0
    c["BCOLS"] = c["B"] // 128
    c["BSH"] = c["B"] // c["NC"]
    c["BSHC"] = c["BSH"] // 128
    return c


# --------------------------------------------------------------------------
# wait splitting post-pass (walrus: max 1 sync wait per instruction)
# --------------------------------------------------------------------------

def split_multi_waits(nc, max_waits=1):
    n = 0
    for f in nc.m.functions:
        for b in f.blocks:
            insts = b.instructions
            items = list(insts)
            out = []
            for i in items:
                si = i.sync_info
                w = list(si.on_wait) if si and si.on_wait else []
                if len(w) > max_waits:
                    for x in w[:-max_waits]:
                        n += 1
                        out.append(mybir.InstNoOp(
                            name=f"waitsplit-{n}",
                            sync_info=mybir.SyncInfo(on_wait=[x], on_update=[]),
                            engine=i.engine, bass_nofuse=True))
                    si.on_wait = w[-max_waits:]
                out.append(i)
            insts.clear()
            insts.extend(out)
    return n


# --------------------------------------------------------------------------
# host prep
# --------------------------------------------------------------------------

def host_prep(inputs, c):
    N, NC, SHARD, SPAD, PC, D = (c["N"], c["NC"], c["SHARD"], c["SPAD"],
                                 c["PC"], c["D"])
    h = np.asarray(inputs["h_list"]).astype(np.int64)
    t = np.asarray(inputs["t_list"]).astype(np.int64)

    deg = np.bincount(h, minlength=N).astype(np.int64)
    with np.errstate(divide="ignore"):
        d_inv = (deg.astype(np.float64) ** -0.5).astype(np.float32)

    perm_row = np.empty(N, dtype=np.int64)
    inv_order = []
    for k in range(NC):
        lo = k * SHARD
        order = np.argsort(deg[lo:lo + SHARD], kind="stable")
        perm_row[lo + order] = k * SPAD + np.arange(SHARD)
        inv_order.append(lo + order)

    dest_pos = perm_row[h]
    eorder = np.argsort(dest_pos, kind="stable")
    dpos_s = dest_pos[eorder]
    src_s = perm_row[t[eorder]]

    ego = np.concatenate([
        np.asarray(inputs["user_emb"], dtype=np.float32),
        np.asarray(inputs["item_emb"], dtype=np.float32),
    ], axis=0)

    # per-core per-tile slot columns
    core_cols = []       # list of dict[(tau, s)] -> int32[128]
    core_smax = []       # per core: [PC] slot counts
    core_zero = []       # per core: [PC] tile has a zero-degree dest
    for k in range(NC):
        base = k * SPAD
        lo_i = np.searchsorted(dpos_s, base)
        hi_i = np.searchsorted(dpos_s, base + SHARD)
        dj = dpos_s[lo_i:hi_i] - base
        sj = src_s[lo_i:hi_i]
        degl = np.zeros(SPAD, dtype=np.int64)
        np.add.at(degl, dj, 1)
        starts = np.zeros(SPAD + 1, dtype=np.int64)
        np.cumsum(degl, out=starts[1:])
        cols = {}
        smax = np.zeros(PC, dtype=np.int64)
        zero = np.zeros(PC, dtype=bool)
        for tau in range(PC):
            jlo = tau * 128
            dtile = degl[jlo:jlo + 128]
            smax[tau] = int(dtile.max())
            zero[tau] = bool((dtile == 0).any())
            for s in range(smax[tau]):
                col = np.full(128, SENT, dtype=np.int64)
                sel = dtile > s
                col[sel] = sj[starts[jlo:jlo + 128][sel] + s]
                cols[(tau, s)] = col.astype(np.int32)
        core_cols.append(cols)
        core_smax.append(smax)
        core_zero.append(zero)

    # SPMD union plan
    smax_u = np.max(np.stack(core_smax), axis=0)
    zero_u = np.any(np.stack(core_zero), axis=0)
    uplan = []
    for tau in range(PC):
        s = int(smax_u[tau])
        if s == 0:
            uplan.append(dict(tau=tau, memset=True, ops=[]))
        elif zero_u[tau]:
            uplan.append(dict(tau=tau, memset=True, ops=["add"] * s))
        else:
            uplan.append(dict(tau=tau, memset=False,
                              ops=["bypass"] + ["add"] * (s - 1)))
    NI = max(1, int(smax_u.sum()))

    per_core = []
    for k in range(NC):
        out_cols = []
        for e in uplan:
            for s in range(len(e["ops"])):
                col = core_cols[k].get((e["tau"], s))
                if col is None:
                    col = np.full(128, SENT, dtype=np.int32)
                out_cols.append(col)
        idx_spmm = (np.stack(out_cols, axis=1) if out_cols
                    else np.zeros((128, 1), np.int32))

        def cyc(vec):
            return vec.reshape(PC, 128).T.copy()

        dloc = np.zeros(SPAD, dtype=np.float32)
        dloc[:SHARD] = d_inv[inv_order[k]]
        mask = np.zeros(SPAD, dtype=np.float32)
        mask[:SHARD] = 1.0
        egp = np.zeros((SPAD, D), dtype=np.float32)
        egp[:SHARD] = ego[inv_order[k]]
        per_core.append(dict(
            idx_spmm=idx_spmm, dinv=cyc(dloc), dinv2=cyc(dloc * dloc),
            kmask=cyc(mask), ego_perm=egp))

    return per_core, ego, perm_row, uplan, NI


# --------------------------------------------------------------------------
# device program
# --------------------------------------------------------------------------

def build_bass(c):
    NC, D, PC, SPAD, TROWS = c["NC"], c["D"], c["PC"], c["SPAD"], c["TROWS"]
    BC, BSHC, NI = c["BCOLS"], c["BSHC"], c["NI"]
    NINT, TS, NL = c["N_INTENTS"], c["T_SIZE"], c["N_LAYERS"]
    TEMP = c["TEMP"]
    uplan = c["UPLAN"]
    NB = 2 * BC + 8 * BSHC

    nc = bass.Bass(num_devices=NC)

    ego_perm = nc.dram_tensor("ego_perm", [SPAD, D], F32, kind="ExternalInput")
    idx_spmm = nc.dram_tensor("idx_spmm", [128, NI], I32, kind="ExternalInput")
    dinv_in = nc.dram_tensor("dinv", [128, PC], F32, kind="ExternalInput")
    dinv2_in = nc.dram_tensor("dinv2", [128, PC], F32, kind="ExternalInput")
    kmask_in = nc.dram_tensor("kmask", [128, PC], F32, kind="ExternalInput")
    idx_b_in = nc.dram_tensor("idx_b", [128, NB], I32, kind="ExternalInput")
    eps_in = nc.dram_tensor("eps", [c["N"], D], F32, kind="ExternalInput")
    ego_full = nc.dram_tensor("ego_full", [c["N"], D], F32,
                              kind="ExternalInput")
    ui_in = nc.dram_tensor("user_intent", [D, NINT], F32, kind="ExternalInput")
    ii_in = nc.dram_tensor("item_intent", [D, NINT], F32, kind="ExternalInput")
    lw_in = nc.dram_tensor("lin_w", [D, TS], F32, kind="ExternalInput")
    lb_in = nc.dram_tensor("lin_b_rep", [128, D], F32, kind="ExternalInput")

    partials = nc.dram_tensor("partials", [1, 16], F32, kind="ExternalOutput")

    BF16 = mybir.dt.bfloat16
    yA = nc.dram_tensor("yA", [TROWS, D], BF16, addr_space="Shared")
    yB = nc.dram_tensor("yB", [TROWS, D], BF16, addr_space="Shared")
    all_emb = nc.dram_tensor("all_emb", [TROWS, D], F32, addr_space="Shared")
    shard_buf = nc.dram_tensor("shard_buf", [SPAD, D], F32)
    shard_bf = nc.dram_tensor("shard_bf", [SPAD, D], BF16)

    groups = [list(range(NC))]

    # batch idx column offsets
    OFF_IU, OFF_IP = 0, BC
    OFF_US = 2 * BC                 # ua shard (perm)
    OFF_PS = OFF_US + BSHC          # ia pos shard (perm)
    OFF_NS = OFF_US + 2 * BSHC      # ia neg shard (perm)
    OFF_EU = OFF_US + 3 * BSHC      # eps users shard (orig)
    OFF_EP = OFF_US + 4 * BSHC      # eps pos shard (orig)
    OFF_GU = OFF_US + 5 * BSHC      # ego users shard (orig)
    OFF_GP = OFF_US + 6 * BSHC
    OFF_GN = OFF_US + 7 * BSHC

    with tile.TileContext(nc) as tc:
        with tc.tile_pool(name="const", bufs=1) as cp, \
             tc.tile_pool(name="work", bufs=2) as wp, \
             tc.tile_pool(name="spmm", bufs=c["PC"] + 8) as sp, \
             tc.tile_pool(name="curp", bufs=8) as curp, \
             tc.tile_pool(name="psum", bufs=1, space="PSUM") as pp:

            ident = cp.tile([128, 128], F32)
            make_identity(nc, ident[:])
            bc_reg = nc.alloc_register(mybir.EngineType.Pool, "bcreg")
            nc.gpsimd.reg_mov(bc_reg, TROWS - 1)
            ones_col = cp.tile([128, 1], F32)
            nc.vector.memset(ones_col[:], 1.0)

            def load(shape, dt, src, name):
                t_ = cp.tile(shape, dt, tag=name)
                nc.sync.dma_start(out=t_[:], in_=src)
                return t_

            idxs = load([128, NI], I32, idx_spmm[:], "idxs")
            dinv = load([128, PC], F32, dinv_in[:], "dinv")
            dinv2 = load([128, PC], F32, dinv2_in[:], "dinv2")
            kmask = load([128, PC], F32, kmask_in[:], "kmask")
            idxb = load([128, NB], I32, idx_b_in[:], "idxb")
            lb_rep = load([128, D], F32, lb_in[:], "lb")
            ui_sb = load([D, NINT], F32, ui_in[:], "ui")
            ii_sb = load([D, NINT], F32, ii_in[:], "ii")
            lw_sb = load([D, TS], F32, lw_in[:], "lw")

            def transpose128(dst_ap, src_ap, P, Fr):
                # src [P, Fr] -> dst [Fr, P]
                ps = pp.tile([128, 128], F32, tag="tps")
                nc.tensor.transpose(out=ps[:Fr, :P], in_=src_ap,
                                    identity=ident[:P, :P])
                nc.vector.tensor_copy(dst_ap, ps[:Fr, :P])

            uiT = cp.tile([NINT, D], F32)
            transpose128(uiT[:], ui_sb[:], D, NINT)
            iiT = cp.tile([NINT, D], F32)
            transpose128(iiT[:], ii_sb[:], D, NINT)
            lwT = cp.tile([TS, D], F32)
            transpose128(lwT[:], lw_sb[:], D, TS)

            acc = cp.tile([128, PC * D], F32)
            nc.sync.dma_start(
                out=acc[:].rearrange("p (c d) -> p c d", d=D),
                in_=ego_perm[:].rearrange("(c p) d -> p c d", p=128))

            def dcol(tbl, tau):
                return tbl[:, tau:tau + 1].to_broadcast([128, D])

            def sbcol(tau):
                return shard_bf[:].rearrange(
                    "(c p) d -> p c d", p=128)[:, tau, :]

            # y0 = d_inv * ego, per column (cast to bf16 for the y table)
            for tau in range(PC):
                yt = sp.tile([128, D], F32, tag="out_t")
                nc.vector.tensor_tensor(
                    out=yt[:], in0=acc[:, tau * D:(tau + 1) * D],
                    in1=dcol(dinv, tau), op=ALU.mult)
                yb = curp.tile([128, D], BF16, tag="ybf")
                nc.vector.tensor_copy(yb[:], yt[:])
                nc.sync.dma_start(out=sbcol(tau), in_=yb[:])
            nc.gpsimd.collective_compute(
                "AllGather", ALU.bypass, replica_groups=groups,
                ins=[shard_bf[:]], outs=[yA[:]])

            smax_all = max((len(e["ops"]) for e in uplan), default=0)
            # column index of instr (tau, s) inside idx_spmm
            colof = {}
            _ic = 0
            for e in uplan:
                for s in range(len(e["ops"])):
                    colof[(e["tau"], s)] = _ic
                    _ic += 1

            ybufs = [yA, yB]
            for layer in range(NL):
                y_in = ybufs[layer % 2]
                y_out = ybufs[(layer + 1) % 2]
                last = (layer == NL - 1)
                outs = {}
                for e in uplan:
                    out_t = sp.tile([128, D], F32, tag="out_t")
                    outs[e["tau"]] = out_t
                    if e["memset"]:
                        nc.vector.memset(out_t[:], 0.0)
                # slot-major round-robin: chain neighbors are ~PC apart in
                # the Pool stream, so DMA completions overlap fully
                for s in range(smax_all):
                    for e in uplan:
                        if s >= len(e["ops"]):
                            continue
                        op = e["ops"][s]
                        kw = {}
                        if op != "bypass":
                            kw = dict(bounds_check=bc_reg,
                                      oob_is_err=False, compute_op=ALU.add)
                        icol = colof[(e["tau"], s)]
                        nc.gpsimd.indirect_dma_start(
                            out=outs[e["tau"]][:], out_offset=None,
                            in_=y_in[:],
                            in_offset=bass.IndirectOffsetOnAxis(
                                ap=idxs[:, icol:icol + 1], axis=0),
                            **kw)
                for e in uplan:
                    tau = e["tau"]
                    out_t = outs[tau]
                    aslice = acc[:, tau * D:(tau + 1) * D]
                    cur = curp.tile([128, D], F32, tag="cur")
                    nc.vector.tensor_tensor(out=cur[:], in0=out_t[:],
                                            in1=dcol(dinv, tau), op=ALU.mult)
                    nc.vector.tensor_add(out=aslice, in0=aslice, in1=cur[:])
                    if not last:
                        # in-place: out_t becomes y_next column
                        nc.vector.tensor_tensor(
                            out=out_t[:], in0=out_t[:], in1=dcol(dinv2, tau),
                            op=ALU.mult)
                        yb = curp.tile([128, D], BF16, tag="ybf")
                        nc.vector.tensor_copy(yb[:], out_t[:])
                        nc.sync.dma_start(out=sbcol(tau), in_=yb[:])
                if not last:
                    nc.gpsimd.collective_compute(
                        "AllGather", ALU.bypass, replica_groups=groups,
                        ins=[shard_bf[:]], outs=[y_out[:]])

            nc.sync.dma_start(
                out=shard_buf[:].rearrange("(c p) d -> p c d", p=128),
                in_=acc[:].rearrange("p (c d) -> p c d", d=D))
            nc.gpsimd.collective_compute(
                "AllGather", ALU.bypass, replica_groups=groups,
                ins=[shard_buf[:]], outs=[all_emb[:]])

            # ---------------- downstream ----------------

            def gather1(src, col, dst_ap):
                nc.gpsimd.indirect_dma_start(
                    out=dst_ap, out_offset=None, in_=src[:],
                    in_offset=bass.IndirectOffsetOnAxis(
                        ap=idxb[:, col:col + 1], axis=0))

            def gather_set(src, off, ncols, tag, pool=cp):
                tl = pool.tile([128, ncols * D], F32, tag=tag)
                for q in range(ncols):
                    gather1(src, off + q, tl[:, q * D:(q + 1) * D])
                return tl

            ua_sh = gather_set(all_emb, OFF_US, BSHC, "ua_sh")
            iap_sh = gather_set(all_emb, OFF_PS, BSHC, "iap_sh")
            ian_sh = gather_set(all_emb, OFF_NS, BSHC, "ian_sh")
            eps_u = gather_set(eps_in, OFF_EU, BSHC, "eps_u")
            eps_p = gather_set(eps_in, OFF_EP, BSHC, "eps_p")
            ego_u = gather_set(ego_full, OFF_GU, BSHC, "ego_u")
            ego_p = gather_set(ego_full, OFF_GP, BSHC, "ego_p")
            ego_n = gather_set(ego_full, OFF_GN, BSHC, "ego_n")

            def normalize_rows(x_ap, ncols):
                for q in range(ncols):
                    sl = x_ap[:, q * D:(q + 1) * D]
                    sq = wp.tile([128, D], F32, tag="sqj")
                    ss = wp.tile([128, 1], F32, tag="ssj")
                    nc.scalar.activation(sq[:], sl, ACTF.Square,
                                         accum_out=ss[:])
                    ls = wp.tile([128, 1], F32, tag="rsj")
                    nc.scalar.activation(ls[:], ss[:], ACTF.Ln)
                    rn = wp.tile([128, 1], F32, tag="rnj")
                    nc.scalar.activation(rn[:], ls[:], ACTF.Exp, scale=-0.5)
                    nc.vector.tensor_scalar_mul(sl, sl, rn[:])

            def intent_pipe(gsrc_off, w_sb, wT_sb, tag):
                """Full-batch intent; returns (shard normalized [128,BSHC*D],
                e2T [D, B])."""
                sh_n = cp.tile([128, BSHC * D], F32, tag=f"in_{tag}")
                e2T = cp.tile([D, BC * 128], F32, tag=f"iT_{tag}")
                for q in range(BC):
                    tl = wp.tile([128, D], F32, tag="itl")
                    gather1(all_emb, gsrc_off + q, tl[:])
                    tT = wp.tile([D, 128], F32, tag="tT")
                    transpose128(tT[:], tl[:], 128, D)
                    zp = pp.tile([128, NINT], F32, tag="zp")
                    nc.tensor.matmul(out=zp[:], lhsT=tT[:], rhs=w_sb[:])
                    z = wp.tile([128, NINT], F32, tag="z")
                    nc.vector.tensor_copy(z[:], zp[:])
                    mx = wp.tile([128, 1], F32, tag="mx")
                    nc.vector.tensor_reduce(out=mx[:], in_=z[:], axis=AX.X,
                                            op=ALU.max)
                    nmx = wp.tile([128, 1], F32, tag="nmx")
                    nc.scalar.mul(nmx[:], mx[:], -1.0)
                    ex = wp.tile([128, NINT], F32, tag="ex")
                    se = wp.tile([128, 1], F32, tag="se")
                    nc.scalar.activation(ex[:], z[:], ACTF.Exp, bias=nmx[:],
                                         accum_out=se[:])
                    rse = wp.tile([128, 1], F32, tag="rse")
                    nc.vector.reciprocal(rse[:], se[:])
                    nc.vector.tensor_scalar_mul(ex[:], ex[:], rse[:])
                    exT = wp.tile([NINT, 128], F32, tag="exT")
                    transpose128(exT[:], ex[:], 128, NINT)
                    op_ = pp.tile([128, D], F32, tag="op")
                    nc.tensor.matmul(out=op_[:], lhsT=exT[:], rhs=wT_sb[:])
                    onrm = wp.tile([128, D], F32, tag="onrm")
                    nc.vector.tensor_copy(onrm[:], op_[:])
                    normalize_rows(onrm[:], 1)
                    if q < BSHC:
                        nc.vector.tensor_copy(
                            sh_n[:, q * D:(q + 1) * D], onrm[:])
                    transpose128(e2T[:, q * 128:(q + 1) * 128], onrm[:],
                                 128, D)
                return sh_n, e2T

            u_i_n, u_i_T = intent_pipe(OFF_IU, ui_sb, uiT, "u")
            i_i_n, i_i_T = intent_pipe(OFF_IP, ii_sb, iiT, "i")

            def gen_pipe(mean_tl, eps_tl, tag):
                gen_n = cp.tile([128, BSHC * D], F32, tag=f"gen_{tag}")
                genT = cp.tile([D, BSHC * 128], F32, tag=f"genT_{tag}")
                for q in range(BSHC):
                    msl = mean_tl[:, q * D:(q + 1) * D]
                    sp_t = wp.tile([128, TS], F32, tag="sp_t")
                    nc.scalar.activation(sp_t[:], msl[:, :TS], ACTF.Exp)
                    nc.vector.tensor_scalar_add(sp_t[:], sp_t[:], 1.0)
                    nc.scalar.activation(sp_t[:], sp_t[:], ACTF.Ln)
                    spT = wp.tile([TS, 128], F32, tag="spT")
                    transpose128(spT[:], sp_t[:], 128, TS)
                    stp = pp.tile([128, D], F32, tag="stp")
                    nc.tensor.matmul(out=stp[:], lhsT=spT[:], rhs=lwT[:])
                    std = wp.tile([128, D], F32, tag="std")
                    nc.vector.tensor_add(out=std[:], in0=stp[:],
                                         in1=lb_rep[:])
                    nc.vector.tensor_scalar_add(std[:], std[:], 1e-8)
                    g = gen_n[:, q * D:(q + 1) * D]
                    nc.vector.tensor_tensor(
                        out=g, in0=eps_tl[:, q * D:(q + 1) * D], in1=std[:],
                        op=ALU.mult)
                    nc.vector.tensor_add(out=g, in0=g, in1=msl)
                normalize_rows(gen_n[:], BSHC)
                for q in range(BSHC):
                    transpose128(genT[:, q * 128:(q + 1) * 128],
                                 gen_n[:, q * D:(q + 1) * D], 128, D)
                return gen_n, genT

            u_gen_n, u_gen_T = gen_pipe(ua_sh, eps_u, "gu")
            i_gen_n, i_gen_T = gen_pipe(iap_sh, eps_p, "gi")

            psb = cp.tile([1, 16], F32)
            nc.vector.memset(psb[:], 0.0)

            def part_sum(vec_ap, slot, P=128):
                ps = pp.tile([1, 1], F32, tag="pscal")
                nc.tensor.matmul(out=ps[:], lhsT=vec_ap, rhs=ones_col[:P, :])
                nc.vector.tensor_add(out=psb[:, slot:slot + 1],
                                     in0=psb[:, slot:slot + 1], in1=ps[:])

            def infonce(e1_n, e1_T, e2sh_n, e2_T, slot):
                lgs = wp.tile([128, BSHC], F32, tag="lgs")
                for q in range(BSHC):
                    prod = wp.tile([128, D], F32, tag="prod")
                    nc.vector.tensor_tensor(
                        out=prod[:], in0=e1_n[:, q * D:(q + 1) * D],
                        in1=e2sh_n[:, q * D:(q + 1) * D], op=ALU.mult)
                    pdot = wp.tile([128, 1], F32, tag="pdot")
                    nc.vector.tensor_reduce(out=pdot[:], in_=prod[:],
                                            axis=AX.X, op=ALU.add)
                    pex = wp.tile([128, 1], F32, tag="pex")
                    nc.scalar.activation(pex[:], pdot[:], ACTF.Exp,
                                         scale=1.0 / TEMP)
                    nss = wp.tile([128, BC], F32, tag="nss")
                    for ch in range(BC):
                        zp = pp.tile([128, 128], F32, tag="zneg")
                        nc.tensor.matmul(
                            out=zp[:], lhsT=e1_T[:, q * 128:(q + 1) * 128],
                            rhs=e2_T[:, ch * 128:(ch + 1) * 128])
                        ju = wp.tile([128, 128], F32, tag="ju")
                        nc.scalar.activation(
                            ju[:], zp[:], ACTF.Exp, scale=1.0 / TEMP,
                            accum_out=nss[:, ch:ch + 1])
                    nsum = wp.tile([128, 1], F32, tag="nsum")
                    nc.vector.tensor_reduce(out=nsum[:], in_=nss[:],
                                            axis=AX.X, op=ALU.add)
                    nc.vector.tensor_scalar_add(nsum[:], nsum[:], 1e-8)
                    rn = wp.tile([128, 1], F32, tag="rng")
                    nc.vector.reciprocal(rn[:], nsum[:])
                    qv = wp.tile([128, 1], F32, tag="qv")
                    nc.vector.tensor_tensor(out=qv[:], in0=pex[:], in1=rn[:],
                                            op=ALU.mult)
                    nc.vector.tensor_scalar_add(qv[:], qv[:], 1e-8)
                    nc.scalar.activation(lgs[:, q:q + 1], qv[:], ACTF.Ln)
                tot = wp.tile([128, 1], F32, tag="lgt")
                nc.vector.tensor_reduce(out=tot[:], in_=lgs[:], axis=AX.X,
                                        op=ALU.add)
                part_sum(tot[:], slot)

            infonce(u_gen_n, u_gen_T, u_i_n, u_i_T, 2)
            infonce(i_gen_n, i_gen_T, i_i_n, i_i_T, 3)

            # BPR
            dsc = wp.tile([128, BSHC], F32, tag="dsc")
            for q in range(BSHC):
                pr = wp.tile([128, D], F32, tag="bprp")
                nc.vector.tensor_tensor(
                    out=pr[:], in0=ua_sh[:, q * D:(q + 1) * D],
                    in1=iap_sh[:, q * D:(q + 1) * D], op=ALU.mult)
                ps_ = wp.tile([128, 1], F32, tag="bps")
                nc.vector.tensor_reduce(out=ps_[:], in_=pr[:], axis=AX.X,
                                        op=ALU.add)
                nr = wp.tile([128, D], F32, tag="bprn")
                nc.vector.tensor_tensor(
                    out=nr[:], in0=ua_sh[:, q * D:(q + 1) * D],
                    in1=ian_sh[:, q * D:(q + 1) * D], op=ALU.mult)
                ns_ = wp.tile([128, 1], F32, tag="bns")
                nc.vector.tensor_reduce(out=ns_[:], in_=nr[:], axis=AX.X,
                                        op=ALU.add)
                nc.vector.tensor_sub(out=dsc[:, q:q + 1], in0=ns_[:],
                                     in1=ps_[:])
            spl = wp.tile([128, BSHC], F32, tag="spl")
            nc.scalar.activation(spl[:], dsc[:], ACTF.Exp)
            nc.vector.tensor_scalar_add(spl[:], spl[:], 1.0)
            nc.scalar.activation(spl[:], spl[:], ACTF.Ln)
            bps = wp.tile([128, 1], F32, tag="bpst")
            nc.vector.tensor_reduce(out=bps[:], in_=spl[:], axis=AX.X,
                                    op=ALU.add)
            part_sum(bps[:], 0)

            # emb partial
            esums = wp.tile([128, 4], F32, tag="esums")
            for j, tl in enumerate([ego_u, ego_p, ego_n]):
                jk = wp.tile([128, BSHC * D], F32, tag="jnk")
                nc.scalar.activation(jk[:], tl[:], ACTF.Square,
                                     accum_out=esums[:, j:j + 1])
            nc.vector.memset(esums[:, 3:4], 0.0)
            etot = wp.tile([128, 1], F32, tag="etot")
            nc.vector.tensor_reduce(out=etot[:], in_=esums[:], axis=AX.X,
                                    op=ALU.add)
            part_sum(etot[:], 4)

            # int partial
            isums = wp.tile([D, 2], F32, tag="isums")
            for j, tl in enumerate([ui_sb, ii_sb]):
                jk2 = wp.tile([D, NINT], F32, tag="jnk2")
                nc.scalar.activation(jk2[:], tl[:], ACTF.Square,
                                     accum_out=isums[:, j:j + 1])
            itot = wp.tile([D, 1], F32, tag="itot")
            nc.vector.tensor_reduce(out=itot[:], in_=isums[:], axis=AX.X,
                                    op=ALU.add)
            part_sum(itot[:], 5, P=D)

            # KL over own shard (from acc)
            KW = 8
            klcols = cp.tile([128, PC], F32)
            for g in range(math.ceil(PC / KW)):
                w0 = g * KW
                W = min(KW, PC - w0)
                mean_g = acc[:, w0 * D:(w0 + W) * D]
                spg = wp.tile([128, KW * TS], F32, tag="spg")
                nc.scalar.activation(
                    spg[:, :W * TS].rearrange("p (c d) -> p c d", d=TS),
                    acc[:, w0 * D:].rearrange(
                        "p (c d) -> p c d", d=D)[:, 0:W, 0:TS],
                    ACTF.Exp)
                nc.vector.tensor_scalar_add(spg[:, :W * TS], spg[:, :W * TS],
                                            1.0)
                nc.scalar.activation(spg[:, :W * TS], spg[:, :W * TS],
                                     ACTF.Ln)
                stdg = wp.tile([128, KW * D], F32, tag="stdg")
                for w in range(W):
                    spT = wp.tile([TS, 128], F32, tag="spTk")
                    transpose128(spT[:], spg[:, w * TS:(w + 1) * TS], 128, TS)
                    stp = pp.tile([128, D], F32, tag="stp")
                    nc.tensor.matmul(out=stp[:], lhsT=spT[:], rhs=lwT[:])
                    sw = stdg[:, w * D:(w + 1) * D]
                    nc.vector.tensor_add(out=sw, in0=stp[:], in1=lb_rep[:])
                    nc.vector.tensor_scalar_add(sw, sw, 1e-8)
                m2 = wp.tile([128, KW * D], F32, tag="m2")
                nc.scalar.activation(m2[:, :W * D], mean_g, ACTF.Square)
                exg = wp.tile([128, KW * D], F32, tag="exg")
                nc.scalar.activation(exg[:, :W * D], stdg[:, :W * D],
                                     ACTF.Exp, scale=2.0)
                t1 = wp.tile([128, KW * D], F32, tag="t1")
                nc.scalar.activation(t1[:, :W * D], stdg[:, :W * D],
                                     ACTF.Copy, bias=0.0, scale=2.0)
                nc.vector.tensor_scalar_add(t1[:, :W * D], t1[:, :W * D], 1.0)
                nc.vector.tensor_sub(out=t1[:, :W * D], in0=t1[:, :W * D],
                                     in1=m2[:, :W * D])
                nc.vector.tensor_sub(out=t1[:, :W * D], in0=t1[:, :W * D],
                                     in1=exg[:, :W * D])
                nc.vector.tensor_reduce(
                    out=klcols[:, w0:w0 + W],
                    in_=t1[:, :W * D].rearrange("p (c d) -> p c d", d=D),
                    axis=AX.X, op=ALU.add)
            nc.vector.tensor_tensor(out=klcols[:], in0=klcols[:],
                                    in1=kmask[:], op=ALU.mult)
            ktot = wp.tile([128, 1], F32, tag="ktot")
            nc.vector.tensor_reduce(out=ktot[:], in_=klcols[:], axis=AX.X,
                                    op=ALU.add)
            part_sum(ktot[:], 1)

            nc.sync.dma_start(out=partials[:], in_=psb[:])

    return nc


# --------------------------------------------------------------------------
# entry
# --------------------------------------------------------------------------

def prepare(inputs, c):
    """Returns (nc, in_maps)."""
    NC = c["NC"]
    per_core, ego, perm_row, uplan, NI = host_prep(inputs, c)
    c["UPLAN"] = uplan
    c["NI"] = NI

    users0 = np.asarray(inputs["users"]).astype(np.int64)
    pos0 = np.asarray(inputs["pos_items"]).astype(np.int64)
    neg0 = np.asarray(inputs["neg_items"]).astype(np.int64)
    N_USERS, B, BSH = c["N_USERS"], c["B"], c["BSH"]

    def cycb(v):
        m = len(v) // 128
        return v.reshape(m, 128).T.astype(np.int32)

    eps_np = np.asarray(inputs["eps"], dtype=np.float32)
    ui_np = np.asarray(inputs["user_intent"], dtype=np.float32)
    ii_np = np.asarray(inputs["item_intent"], dtype=np.float32)
    lw_np = np.asarray(inputs["lin_w"], dtype=np.float32)
    lb_rep = np.tile(np.asarray(inputs["lin_b"],
                                dtype=np.float32)[None, :], (128, 1))

    in_maps = []
    for k in range(NC):
        rot = np.roll(np.arange(B), -k * BSH)
        users, pos, neg = users0[rot], pos0[rot], neg0[rot]
        sh = slice(0, BSH)
        idx_b = np.concatenate([
            cycb(perm_row[users]),                    # OFF_IU (full)
            cycb(perm_row[N_USERS + pos]),            # OFF_IP (full)
            cycb(perm_row[users[sh]]),                # OFF_US
            cycb(perm_row[N_USERS + pos[sh]]),        # OFF_PS
            cycb(perm_row[N_USERS + neg[sh]]),        # OFF_NS
            cycb(users[sh]),                          # OFF_EU
            cycb(N_USERS + pos[sh]),                  # OFF_EP
            cycb(users[sh]),                          # OFF_GU
            cycb(N_USERS + pos[sh]),                  # OFF_GP
            cycb(N_USERS + neg[sh]),                  # OFF_GN
        ], axis=1)
        pk = per_core[k]
        in_maps.append(dict(
            ego_perm=pk["ego_perm"], idx_spmm=pk["idx_spmm"],
            dinv=pk["dinv"], dinv2=pk["dinv2"], kmask=pk["kmask"],
            idx_b=idx_b, eps=eps_np, ego_full=ego, user_intent=ui_np,
            item_intent=ii_np, lin_w=lw_np, lin_b_rep=lb_rep))

    nc = build_bass(c)
    split_multi_waits(nc)
    return nc, in_maps


def combine(results, c):
    NC, B, N = c["NC"], c["B"], c["N"]
    P = np.stack([np.asarray(results[k]["partials"][0], dtype=np.float64)
                  for k in range(NC)])
    bpr = P[:, 0].sum() / B
    kl = c["KL_REG"] * (-0.5 * P[:, 1].sum()) / N
    gen_loss = np.float32(bpr + kl)
    cl_loss = np.float32(c["SSL_REG"] * (-(P[:, 2].sum()) - P[:, 3].sum()) / B)
    emb_loss = np.float32(c["EMB_REG"] * P[:, 4].sum())
    int_loss = np.float32(c["INT_REG"] * P[0, 5])
    return (gen_loss, cl_loss, emb_loss, int_loss)


def kernel(**inputs):
    c = derive(default_cfg())
    nc, in_maps = prepare(inputs, c)
    res = run_bass_kernel_spmd(nc, in_maps, list(range(c["NC"])))
    return combine(res.results, c)



# revision 24
# speedup vs baseline: 1.0820x; 1.0660x over previous
"""DVGCL (GNN message passing + contrastive losses) on 8 Trainium2 cores.

Sharding: node dim N split 8 ways by destination; each shard degree-sorted and
laid out cyclically (pos j -> partition j%128, col j//128); the permutation is
folded into every index array on the host. The symmetric normalization
separates (g = d_inv[h] d_inv[t]), so propagation gathers the pre-scaled table
y = d_inv * cur unweighted and rescales shard outputs. Per 128-dest tile,
slot-wise indirect_dma_start with compute_op=add accumulates gathered rows in
SBUF via the DMA CCE units; sentinel indices are skipped via bounds_check.
Layers are separated by AllGather of the 8 shard updates (double-buffered
table). Losses are computed as per-core partials (batch rotated per core so
its 1/8 slice is always at columns [0, BSHC)) and combined on the host.

Walrus codegen accepts at most ONE sync wait per instruction, so
split_multi_waits hoists extras onto same-engine NoOps after Tile scheduling.
"""
import math
import numpy as np

import concourse.bass as bass
import concourse.mybir as mybir
import concourse.tile as tile
from concourse.bass_utils import run_bass_kernel_spmd
from concourse.masks import make_identity

F32 = mybir.dt.float32
I32 = mybir.dt.int32
AX = mybir.AxisListType
ALU = mybir.AluOpType
ACTF = mybir.ActivationFunctionType

SENT = 1 << 20


def default_cfg():
    return dict(
        N_USERS=50000, N_ITEMS=100000, D=64, N_LAYERS=3, N_INTENTS=128,
        T_SIZE=32, TEMP=0.2, KL_REG=0.01, EMB_REG=1e-5, INT_REG=1e-5,
        SSL_REG=0.1, B=4096, NC=8,
    )


def derive(cfg):
    c = dict(cfg)
    c["N"] = c["N_USERS"] + c["N_ITEMS"]
    assert c["N"] % c["NC"] == 0
    c["SHARD"] = c["N"] // c["NC"]
    c["PC"] = math.ceil(c["SHARD"] / 128)
    c["SPAD"] = 128 * c["PC"]
    c["TROWS"] = c["NC"] * c["SPAD"]
    assert c["B"] % 128 == 0 and (c["B"] // c["NC"]) % 128 == 0
    c["BCOLS"] = c["B"] // 128
    c["BSH"] = c["B"] // c["NC"]
    c["BSHC"] = c["BSH"] // 128
    return c


# --------------------------------------------------------------------------
# wait splitting post-pass (walrus: max 1 sync wait per instruction)
# --------------------------------------------------------------------------

def split_multi_waits(nc, max_waits=1):
    n = 0
    for f in nc.m.functions:
        for b in f.blocks:
            insts = b.instructions
            items = list(insts)
            out = []
            for i in items:
                si = i.sync_info
                w = list(si.on_wait) if si and si.on_wait else []
                if len(w) > max_waits:
                    for x in w[:-max_waits]:
                        n += 1
                        out.append(mybir.InstNoOp(
                            name=f"waitsplit-{n}",
                            sync_info=mybir.SyncInfo(on_wait=[x], on_update=[]),
                            engine=i.engine, bass_nofuse=True))
                    si.on_wait = w[-max_waits:]
                out.append(i)
            insts.clear()
            insts.extend(out)
    return n


# --------------------------------------------------------------------------
# host prep
# --------------------------------------------------------------------------

def host_prep(inputs, c):
    N, NC, SHARD, SPAD, PC, D = (c["N"], c["NC"], c["SHARD"], c["SPAD"],
                                 c["PC"], c["D"])
    h = np.asarray(inputs["h_list"]).astype(np.int64)
    t = np.asarray(inputs["t_list"]).astype(np.int64)

    deg = np.bincount(h, minlength=N).astype(np.int64)
    with np.errstate(divide="ignore"):
        d_inv = (deg.astype(np.float64) ** -0.5).astype(np.float32)

    perm_row = np.empty(N, dtype=np.int64)
    inv_order = []
    for k in range(NC):
        lo = k * SHARD
        order = np.argsort(deg[lo:lo + SHARD], kind="stable")
        perm_row[lo + order] = k * SPAD + np.arange(SHARD)
        inv_order.append(lo + order)

    dest_pos = perm_row[h]
    eorder = np.argsort(dest_pos, kind="stable")
    dpos_s = dest_pos[eorder]
    src_s = perm_row[t[eorder]]

    ego = np.concatenate([
        np.asarray(inputs["user_emb"], dtype=np.float32),
        np.asarray(inputs["item_emb"], dtype=np.float32),
    ], axis=0)

    # per-core per-tile slot columns
    core_cols = []       # list of dict[(tau, s)] -> int32[128]
    core_smax = []       # per core: [PC] slot counts
    core_zero = []       # per core: [PC] tile has a zero-degree dest
    for k in range(NC):
        base = k * SPAD
        lo_i = np.searchsorted(dpos_s, base)
        hi_i = np.searchsorted(dpos_s, base + SHARD)
        dj = dpos_s[lo_i:hi_i] - base
        sj = src_s[lo_i:hi_i]
        degl = np.zeros(SPAD, dtype=np.int64)
        np.add.at(degl, dj, 1)
        starts = np.zeros(SPAD + 1, dtype=np.int64)
        np.cumsum(degl, out=starts[1:])
        cols = {}
        smax = np.zeros(PC, dtype=np.int64)
        zero = np.zeros(PC, dtype=bool)
        for tau in range(PC):
            jlo = tau * 128
            dtile = degl[jlo:jlo + 128]
            smax[tau] = int(dtile.max())
            zero[tau] = bool((dtile == 0).any())
            for s in range(smax[tau]):
                col = np.full(128, SENT, dtype=np.int64)
                sel = dtile > s
                col[sel] = sj[starts[jlo:jlo + 128][sel] + s]
                cols[(tau, s)] = col.astype(np.int32)
        core_cols.append(cols)
        core_smax.append(smax)
        core_zero.append(zero)

    # SPMD union plan
    smax_u = np.max(np.stack(core_smax), axis=0)
    zero_u = np.any(np.stack(core_zero), axis=0)
    uplan = []
    for tau in range(PC):
        s = int(smax_u[tau])
        if s == 0:
            uplan.append(dict(tau=tau, memset=True, ops=[]))
        elif zero_u[tau]:
            uplan.append(dict(tau=tau, memset=True, ops=["add"] * s))
        else:
            uplan.append(dict(tau=tau, memset=False,
                              ops=["bypass"] + ["add"] * (s - 1)))
    NI = max(1, int(smax_u.sum()))

    per_core = []
    for k in range(NC):
        out_cols = []
        for e in uplan:
            for s in range(len(e["ops"])):
                col = core_cols[k].get((e["tau"], s))
                if col is None:
                    col = np.full(128, SENT, dtype=np.int32)
                out_cols.append(col)
        idx_spmm = (np.stack(out_cols, axis=1) if out_cols
                    else np.zeros((128, 1), np.int32))

        def cyc(vec):
            return vec.reshape(PC, 128).T.copy()

        dloc = np.zeros(SPAD, dtype=np.float32)
        dloc[:SHARD] = d_inv[inv_order[k]]
        mask = np.zeros(SPAD, dtype=np.float32)
        mask[:SHARD] = 1.0
        egp = np.zeros((SPAD, D), dtype=np.float32)
        egp[:SHARD] = ego[inv_order[k]]
        per_core.append(dict(
            idx_spmm=idx_spmm, dinv=cyc(dloc), dinv2=cyc(dloc * dloc),
            kmask=cyc(mask), ego_perm=egp))

    return per_core, ego, perm_row, uplan, NI


# --------------------------------------------------------------------------
# device program
# --------------------------------------------------------------------------

def build_bass(c):
    NC, D, PC, SPAD, TROWS = c["NC"], c["D"], c["PC"], c["SPAD"], c["TROWS"]
    BC, BSHC, NI = c["BCOLS"], c["BSHC"], c["NI"]
    NINT, TS, NL = c["N_INTENTS"], c["T_SIZE"], c["N_LAYERS"]
    TEMP = c["TEMP"]
    uplan = c["UPLAN"]
    NB = 2 * BC + 8 * BSHC

    nc = bass.Bass(num_devices=NC)

    ego_perm = nc.dram_tensor("ego_perm", [SPAD, D], F32, kind="ExternalInput")
    idx_spmm = nc.dram_tensor("idx_spmm", [128, NI], I32, kind="ExternalInput")
    dinv_in = nc.dram_tensor("dinv", [128, PC], F32, kind="ExternalInput")
    dinv2_in = nc.dram_tensor("dinv2", [128, PC], F32, kind="ExternalInput")
    kmask_in = nc.dram_tensor("kmask", [128, PC], F32, kind="ExternalInput")
    idx_b_in = nc.dram_tensor("idx_b", [128, NB], I32, kind="ExternalInput")
    eps_in = nc.dram_tensor("eps", [c["N"], D], F32, kind="ExternalInput")
    ego_full = nc.dram_tensor("ego_full", [c["N"], D], F32,
                              kind="ExternalInput")
    ui_in = nc.dram_tensor("user_intent", [D, NINT], F32, kind="ExternalInput")
    ii_in = nc.dram_tensor("item_intent", [D, NINT], F32, kind="ExternalInput")
    lw_in = nc.dram_tensor("lin_w", [D, TS], F32, kind="ExternalInput")
    lb_in = nc.dram_tensor("lin_b_rep", [128, D], F32, kind="ExternalInput")

    partials = nc.dram_tensor("partials", [1, 16], F32, kind="ExternalOutput")

    BF16 = mybir.dt.bfloat16
    yA = nc.dram_tensor("yA", [TROWS, D], BF16, addr_space="Shared")
    yB = nc.dram_tensor("yB", [TROWS, D], BF16, addr_space="Shared")
    all_emb = nc.dram_tensor("all_emb", [TROWS, D], F32, addr_space="Shared")
    shard_buf = nc.dram_tensor("shard_buf", [SPAD, D], F32)
    shard_bf = nc.dram_tensor("shard_bf", [SPAD, D], BF16)

    groups = [list(range(NC))]

    # batch idx column offsets
    OFF_IU, OFF_IP = 0, BC
    OFF_US = 2 * BC                 # ua shard (perm)
    OFF_PS = OFF_US + BSHC          # ia pos shard (perm)
    OFF_NS = OFF_US + 2 * BSHC      # ia neg shard (perm)
    OFF_EU = OFF_US + 3 * BSHC      # eps users shard (orig)
    OFF_EP = OFF_US + 4 * BSHC      # eps pos shard (orig)
    OFF_GU = OFF_US + 5 * BSHC      # ego users shard (orig)
    OFF_GP = OFF_US + 6 * BSHC
    OFF_GN = OFF_US + 7 * BSHC

    with tile.TileContext(nc) as tc:
        with tc.tile_pool(name="const", bufs=1) as cp, \
             tc.tile_pool(name="work", bufs=2) as wp, \
             tc.tile_pool(name="spmm", bufs=36) as sp, \
             tc.tile_pool(name="curp", bufs=8) as curp, \
             tc.tile_pool(name="psum", bufs=1, space="PSUM") as pp:

            ident = cp.tile([128, 128], F32)
            make_identity(nc, ident[:])
            bc_reg = nc.alloc_register(mybir.EngineType.Pool, "bcreg")
            nc.gpsimd.reg_mov(bc_reg, TROWS - 1)
            ones_col = cp.tile([128, 1], F32)
            nc.vector.memset(ones_col[:], 1.0)

            def load(shape, dt, src, name):
                t_ = cp.tile(shape, dt, tag=name)
                nc.sync.dma_start(out=t_[:], in_=src)
                return t_

            idxs = load([128, NI], I32, idx_spmm[:], "idxs")
            dinv = load([128, PC], F32, dinv_in[:], "dinv")
            dinv2 = load([128, PC], F32, dinv2_in[:], "dinv2")
            kmask = load([128, PC], F32, kmask_in[:], "kmask")
            idxb = load([128, NB], I32, idx_b_in[:], "idxb")
            lb_rep = load([128, D], F32, lb_in[:], "lb")
            ui_sb = load([D, NINT], F32, ui_in[:], "ui")
            ii_sb = load([D, NINT], F32, ii_in[:], "ii")
            lw_sb = load([D, TS], F32, lw_in[:], "lw")

            def transpose128(dst_ap, src_ap, P, Fr):
                # src [P, Fr] -> dst [Fr, P]
                ps = pp.tile([128, 128], F32, tag="tps")
                nc.tensor.transpose(out=ps[:Fr, :P], in_=src_ap,
                                    identity=ident[:P, :P])
                nc.vector.tensor_copy(dst_ap, ps[:Fr, :P])

            uiT = cp.tile([NINT, D], F32)
            transpose128(uiT[:], ui_sb[:], D, NINT)
            iiT = cp.tile([NINT, D], F32)
            transpose128(iiT[:], ii_sb[:], D, NINT)
            lwT = cp.tile([TS, D], F32)
            transpose128(lwT[:], lw_sb[:], D, TS)

            acc = cp.tile([128, PC * D], F32)
            nc.sync.dma_start(
                out=acc[:].rearrange("p (c d) -> p c d", d=D),
                in_=ego_perm[:].rearrange("(c p) d -> p c d", p=128))

            def dcol(tbl, tau):
                return tbl[:, tau:tau + 1].to_broadcast([128, D])

            def sbcol(tau):
                return shard_bf[:].rearrange(
                    "(c p) d -> p c d", p=128)[:, tau, :]

            # y0 = d_inv * ego, per column (cast to bf16 for the y table)
            for tau in range(PC):
                yt = sp.tile([128, D], F32, tag="out_t")
                nc.vector.tensor_tensor(
                    out=yt[:], in0=acc[:, tau * D:(tau + 1) * D],
                    in1=dcol(dinv, tau), op=ALU.mult)
                yb = curp.tile([128, D], BF16, tag="ybf")
                nc.vector.tensor_copy(yb[:], yt[:])
                nc.sync.dma_start(out=sbcol(tau), in_=yb[:])
            nc.gpsimd.collective_compute(
                "AllGather", ALU.bypass, replica_groups=groups,
                ins=[shard_bf[:]], outs=[yA[:]])

            # column index of instr (tau, s) inside idx_spmm
            colof = {}
            _ic = 0
            for e in uplan:
                for s in range(len(e["ops"])):
                    colof[(e["tau"], s)] = _ic
                    _ic += 1

            M = 4   # slots per indirect op (amortizes SWDGE fixed cost)
            W = 16  # tiles per window (bounds live out_t tiles)
            ybufs = [yA, yB]
            for layer in range(NL):
                y_in = ybufs[layer % 2]
                y_out = ybufs[(layer + 1) % 2]
                last = (layer == NL - 1)
                for w0 in range(0, len(uplan), W):
                    win = uplan[w0:w0 + W]
                    outs = {}
                    for e in win:
                        out_t = sp.tile([128, M * D], F32, tag="out_t")
                        outs[e["tau"]] = out_t
                        nc.vector.memset(out_t[:], 0.0)
                    gmax = max((-(-len(e["ops"]) // M) for e in win),
                               default=0)
                    # group-major round-robin across the window so same-tile
                    # CCE-add chains keep ~W ops of slack
                    for g in range(gmax):
                        for e in win:
                            ns = len(e["ops"])
                            if g * M >= ns:
                                continue
                            mw = min(M, ns - g * M)
                            icol = colof[(e["tau"], g * M)]
                            nc.gpsimd.indirect_dma_start(
                                out=outs[e["tau"]][:, :mw * D],
                                out_offset=None,
                                in_=y_in[:],
                                in_offset=bass.IndirectOffsetOnAxis(
                                    ap=idxs[:, icol:icol + mw], axis=0),
                                bounds_check=bc_reg, oob_is_err=False,
                                compute_op=ALU.add)
                    for e in win:
                        tau = e["tau"]
                        out_t = outs[tau]
                        red = curp.tile([128, D], F32, tag="red")
                        nc.vector.tensor_reduce(
                            out=red[:],
                            in_=out_t[:].rearrange("p (m d) -> p d m", d=D),
                            axis=AX.X, op=ALU.add)
                        aslice = acc[:, tau * D:(tau + 1) * D]
                        cur = curp.tile([128, D], F32, tag="cur")
                        nc.vector.tensor_tensor(out=cur[:], in0=red[:],
                                                in1=dcol(dinv, tau),
                                                op=ALU.mult)
                        nc.vector.tensor_add(out=aslice, in0=aslice,
                                             in1=cur[:])
                        if not last:
                            nc.vector.tensor_tensor(
                                out=red[:], in0=red[:], in1=dcol(dinv2, tau),
                                op=ALU.mult)
                            yb = curp.tile([128, D], BF16, tag="ybf")
                            nc.vector.tensor_copy(yb[:], red[:])
                            nc.sync.dma_start(out=sbcol(tau), in_=yb[:])
                if not last:
                    nc.gpsimd.collective_compute(
                        "AllGather", ALU.bypass, replica_groups=groups,
                        ins=[shard_bf[:]], outs=[y_out[:]])

            nc.sync.dma_start(
                out=shard_buf[:].rearrange("(c p) d -> p c d", p=128),
                in_=acc[:].rearrange("p (c d) -> p c d", d=D))
            nc.gpsimd.collective_compute(
                "AllGather", ALU.bypass, replica_groups=groups,
                ins=[shard_buf[:]], outs=[all_emb[:]])

            # ---------------- downstream ----------------

            def gather1(src, col, dst_ap):
                nc.gpsimd.indirect_dma_start(
                    out=dst_ap, out_offset=None, in_=src[:],
                    in_offset=bass.IndirectOffsetOnAxis(
                        ap=idxb[:, col:col + 1], axis=0))

            def gather_set(src, off, ncols, tag, pool=cp):
                tl = pool.tile([128, ncols * D], F32, tag=tag)
                for q in range(ncols):
                    gather1(src, off + q, tl[:, q * D:(q + 1) * D])
                return tl

            ua_sh = gather_set(all_emb, OFF_US, BSHC, "ua_sh")
            iap_sh = gather_set(all_emb, OFF_PS, BSHC, "iap_sh")
            ian_sh = gather_set(all_emb, OFF_NS, BSHC, "ian_sh")
            eps_u = gather_set(eps_in, OFF_EU, BSHC, "eps_u")
            eps_p = gather_set(eps_in, OFF_EP, BSHC, "eps_p")
            ego_u = gather_set(ego_full, OFF_GU, BSHC, "ego_u")
            ego_p = gather_set(ego_full, OFF_GP, BSHC, "ego_p")
            ego_n = gather_set(ego_full, OFF_GN, BSHC, "ego_n")

            def normalize_rows(x_ap, ncols):
                for q in range(ncols):
                    sl = x_ap[:, q * D:(q + 1) * D]
                    sq = wp.tile([128, D], F32, tag="sqj")
                    ss = wp.tile([128, 1], F32, tag="ssj")
                    nc.scalar.activation(sq[:], sl, ACTF.Square,
                                         accum_out=ss[:])
                    ls = wp.tile([128, 1], F32, tag="rsj")
                    nc.scalar.activation(ls[:], ss[:], ACTF.Ln)
                    rn = wp.tile([128, 1], F32, tag="rnj")
                    nc.scalar.activation(rn[:], ls[:], ACTF.Exp, scale=-0.5)
                    nc.vector.tensor_scalar_mul(sl, sl, rn[:])

            def intent_pipe(gsrc_off, w_sb, wT_sb, tag):
                """Full-batch intent; returns (shard normalized [128,BSHC*D],
                e2T [D, B])."""
                sh_n = cp.tile([128, BSHC * D], F32, tag=f"in_{tag}")
                e2T = cp.tile([D, BC * 128], F32, tag=f"iT_{tag}")
                for q in range(BC):
                    tl = wp.tile([128, D], F32, tag="itl")
                    gather1(all_emb, gsrc_off + q, tl[:])
                    tT = wp.tile([D, 128], F32, tag="tT")
                    transpose128(tT[:], tl[:], 128, D)
                    zp = pp.tile([128, NINT], F32, tag="zp")
                    nc.tensor.matmul(out=zp[:], lhsT=tT[:], rhs=w_sb[:])
                    z = wp.tile([128, NINT], F32, tag="z")
                    nc.vector.tensor_copy(z[:], zp[:])
                    mx = wp.tile([128, 1], F32, tag="mx")
                    nc.vector.tensor_reduce(out=mx[:], in_=z[:], axis=AX.X,
                                            op=ALU.max)
                    nmx = wp.tile([128, 1], F32, tag="nmx")
                    nc.scalar.mul(nmx[:], mx[:], -1.0)
                    ex = wp.tile([128, NINT], F32, tag="ex")
                    se = wp.tile([128, 1], F32, tag="se")
                    nc.scalar.activation(ex[:], z[:], ACTF.Exp, bias=nmx[:],
                                         accum_out=se[:])
                    rse = wp.tile([128, 1], F32, tag="rse")
                    nc.vector.reciprocal(rse[:], se[:])
                    nc.vector.tensor_scalar_mul(ex[:], ex[:], rse[:])
                    exT = wp.tile([NINT, 128], F32, tag="exT")
                    transpose128(exT[:], ex[:], 128, NINT)
                    op_ = pp.tile([128, D], F32, tag="op")
                    nc.tensor.matmul(out=op_[:], lhsT=exT[:], rhs=wT_sb[:])
                    onrm = wp.tile([128, D], F32, tag="onrm")
                    nc.vector.tensor_copy(onrm[:], op_[:])
                    normalize_rows(onrm[:], 1)
                    if q < BSHC:
                        nc.vector.tensor_copy(
                            sh_n[:, q * D:(q + 1) * D], onrm[:])
                    transpose128(e2T[:, q * 128:(q + 1) * 128], onrm[:],
                                 128, D)
                return sh_n, e2T

            u_i_n, u_i_T = intent_pipe(OFF_IU, ui_sb, uiT, "u")
            i_i_n, i_i_T = intent_pipe(OFF_IP, ii_sb, iiT, "i")

            def gen_pipe(mean_tl, eps_tl, tag):
                gen_n = cp.tile([128, BSHC * D], F32, tag=f"gen_{tag}")
                genT = cp.tile([D, BSHC * 128], F32, tag=f"genT_{tag}")
                for q in range(BSHC):
                    msl = mean_tl[:, q * D:(q + 1) * D]
                    sp_t = wp.tile([128, TS], F32, tag="sp_t")
                    nc.scalar.activation(sp_t[:], msl[:, :TS], ACTF.Exp)
                    nc.vector.tensor_scalar_add(sp_t[:], sp_t[:], 1.0)
                    nc.scalar.activation(sp_t[:], sp_t[:], ACTF.Ln)
                    spT = wp.tile([TS, 128], F32, tag="spT")
                    transpose128(spT[:], sp_t[:], 128, TS)
                    stp = pp.tile([128, D], F32, tag="stp")
                    nc.tensor.matmul(out=stp[:], lhsT=spT[:], rhs=lwT[:])
                    std = wp.tile([128, D], F32, tag="std")
                    nc.vector.tensor_add(out=std[:], in0=stp[:],
                                         in1=lb_rep[:])
                    nc.vector.tensor_scalar_add(std[:], std[:], 1e-8)
                    g = gen_n[:, q * D:(q + 1) * D]
                    nc.vector.tensor_tensor(
                        out=g, in0=eps_tl[:, q * D:(q + 1) * D], in1=std[:],
                        op=ALU.mult)
                    nc.vector.tensor_add(out=g, in0=g, in1=msl)
                normalize_rows(gen_n[:], BSHC)
                for q in range(BSHC):
                    transpose128(genT[:, q * 128:(q + 1) * 128],
                                 gen_n[:, q * D:(q + 1) * D], 128, D)
                return gen_n, genT

            u_gen_n, u_gen_T = gen_pipe(ua_sh, eps_u, "gu")
            i_gen_n, i_gen_T = gen_pipe(iap_sh, eps_p, "gi")

            psb = cp.tile([1, 16], F32)
            nc.vector.memset(psb[:], 0.0)

            def part_sum(vec_ap, slot, P=128):
                ps = pp.tile([1, 1], F32, tag="pscal")
                nc.tensor.matmul(out=ps[:], lhsT=vec_ap, rhs=ones_col[:P, :])
                nc.vector.tensor_add(out=psb[:, slot:slot + 1],
                                     in0=psb[:, slot:slot + 1], in1=ps[:])

            def infonce(e1_n, e1_T, e2sh_n, e2_T, slot):
                lgs = wp.tile([128, BSHC], F32, tag="lgs")
                for q in range(BSHC):
                    prod = wp.tile([128, D], F32, tag="prod")
                    nc.vector.tensor_tensor(
                        out=prod[:], in0=e1_n[:, q * D:(q + 1) * D],
                        in1=e2sh_n[:, q * D:(q + 1) * D], op=ALU.mult)
                    pdot = wp.tile([128, 1], F32, tag="pdot")
                    nc.vector.tensor_reduce(out=pdot[:], in_=prod[:],
                                            axis=AX.X, op=ALU.add)
                    pex = wp.tile([128, 1], F32, tag="pex")
                    nc.scalar.activation(pex[:], pdot[:], ACTF.Exp,
                                         scale=1.0 / TEMP)
                    nss = wp.tile([128, BC], F32, tag="nss")
                    for ch in range(BC):
                        zp = pp.tile([128, 128], F32, tag="zneg")
                        nc.tensor.matmul(
                            out=zp[:], lhsT=e1_T[:, q * 128:(q + 1) * 128],
                            rhs=e2_T[:, ch * 128:(ch + 1) * 128])
                        ju = wp.tile([128, 128], F32, tag="ju")
                        nc.scalar.activation(
                            ju[:], zp[:], ACTF.Exp, scale=1.0 / TEMP,
                            accum_out=nss[:, ch:ch + 1])
                    nsum = wp.tile([128, 1], F32, tag="nsum")
                    nc.vector.tensor_reduce(out=nsum[:], in_=nss[:],
                                            axis=AX.X, op=ALU.add)
                    nc.vector.tensor_scalar_add(nsum[:], nsum[:], 1e-8)
                    rn = wp.tile([128, 1], F32, tag="rng")
                    nc.vector.reciprocal(rn[:], nsum[:])
                    qv = wp.tile([128, 1], F32, tag="qv")
                    nc.vector.tensor_tensor(out=qv[:], in0=pex[:], in1=rn[:],
                                            op=ALU.mult)
                    nc.vector.tensor_scalar_add(qv[:], qv[:], 1e-8)
                    nc.scalar.activation(lgs[:, q:q + 1], qv[:], ACTF.Ln)
                tot = wp.tile([128, 1], F32, tag="lgt")
                nc.vector.tensor_reduce(out=tot[:], in_=lgs[:], axis=AX.X,
                                        op=ALU.add)
                part_sum(tot[:], slot)

            infonce(u_gen_n, u_gen_T, u_i_n, u_i_T, 2)
            infonce(i_gen_n, i_gen_T, i_i_n, i_i_T, 3)

            # BPR
            dsc = wp.tile([128, BSHC], F32, tag="dsc")
            for q in range(BSHC):
                pr = wp.tile([128, D], F32, tag="bprp")
                nc.vector.tensor_tensor(
                    out=pr[:], in0=ua_sh[:, q * D:(q + 1) * D],
                    in1=iap_sh[:, q * D:(q + 1) * D], op=ALU.mult)
                ps_ = wp.tile([128, 1], F32, tag="bps")
                nc.vector.tensor_reduce(out=ps_[:], in_=pr[:], axis=AX.X,
                                        op=ALU.add)
                nr = wp.tile([128, D], F32, tag="bprn")
                nc.vector.tensor_tensor(
                    out=nr[:], in0=ua_sh[:, q * D:(q + 1) * D],
                    in1=ian_sh[:, q * D:(q + 1) * D], op=ALU.mult)
                ns_ = wp.tile([128, 1], F32, tag="bns")
                nc.vector.tensor_reduce(out=ns_[:], in_=nr[:], axis=AX.X,
                                        op=ALU.add)
                nc.vector.tensor_sub(out=dsc[:, q:q + 1], in0=ns_[:],
                                     in1=ps_[:])
            spl = wp.tile([128, BSHC], F32, tag="spl")
            nc.scalar.activation(spl[:], dsc[:], ACTF.Exp)
            nc.vector.tensor_scalar_add(spl[:], spl[:], 1.0)
            nc.scalar.activation(spl[:], spl[:], ACTF.Ln)
            bps = wp.tile([128, 1], F32, tag="bpst")
            nc.vector.tensor_reduce(out=bps[:], in_=spl[:], axis=AX.X,
                                    op=ALU.add)
            part_sum(bps[:], 0)

            # emb partial
            esums = wp.tile([128, 4], F32, tag="esums")
            for j, tl in enumerate([ego_u, ego_p, ego_n]):
                jk = wp.tile([128, BSHC * D], F32, tag="jnk")
                nc.scalar.activation(jk[:], tl[:], ACTF.Square,
                                     accum_out=esums[:, j:j + 1])
            nc.vector.memset(esums[:, 3:4], 0.0)
            etot = wp.tile([128, 1], F32, tag="etot")
            nc.vector.tensor_reduce(out=etot[:], in_=esums[:], axis=AX.X,
                                    op=ALU.add)
            part_sum(etot[:], 4)

            # int partial
            isums = wp.tile([D, 2], F32, tag="isums")
            for j, tl in enumerate([ui_sb, ii_sb]):
                jk2 = wp.tile([D, NINT], F32, tag="jnk2")
                nc.scalar.activation(jk2[:], tl[:], ACTF.Square,
                                     accum_out=isums[:, j:j + 1])
            itot = wp.tile([D, 1], F32, tag="itot")
            nc.vector.tensor_reduce(out=itot[:], in_=isums[:], axis=AX.X,
                                    op=ALU.add)
            part_sum(itot[:], 5, P=D)

            # KL over own shard (from acc)
            KW = 8
            klcols = cp.tile([128, PC], F32)
            for g in range(math.ceil(PC / KW)):
                w0 = g * KW
                W = min(KW, PC - w0)
                mean_g = acc[:, w0 * D:(w0 + W) * D]
                spg = wp.tile([128, KW * TS], F32, tag="spg")
                nc.scalar.activation(
                    spg[:, :W * TS].rearrange("p (c d) -> p c d", d=TS),
                    acc[:, w0 * D:].rearrange(
                        "p (c d) -> p c d", d=D)[:, 0:W, 0:TS],
                    ACTF.Exp)
                nc.vector.tensor_scalar_add(spg[:, :W * TS], spg[:, :W * TS],
                                            1.0)
                nc.scalar.activation(spg[:, :W * TS], spg[:, :W * TS],
                                     ACTF.Ln)
                stdg = wp.tile([128, KW * D], F32, tag="stdg")
                for w in range(W):
                    spT = wp.tile([TS, 128], F32, tag="spTk")
                    transpose128(spT[:], spg[:, w * TS:(w + 1) * TS], 128, TS)
                    stp = pp.tile([128, D], F32, tag="stp")
                    nc.tensor.matmul(out=stp[:], lhsT=spT[:], rhs=lwT[:])
                    sw = stdg[:, w * D:(w + 1) * D]
                    nc.vector.tensor_add(out=sw, in0=stp[:], in1=lb_rep[:])
                    nc.vector.tensor_scalar_add(sw, sw, 1e-8)
                m2 = wp.tile([128, KW * D], F32, tag="m2")
                nc.scalar.activation(m2[:, :W * D], mean_g, ACTF.Square)
                exg = wp.tile([128, KW * D], F32, tag="exg")
                nc.scalar.activation(exg[:, :W * D], stdg[:, :W * D],
                                     ACTF.Exp, scale=2.0)
                t1 = wp.tile([128, KW * D], F32, tag="t1")
                nc.scalar.activation(t1[:, :W * D], stdg[:, :W * D],
                                     ACTF.Copy, bias=0.0, scale=2.0)
                nc.vector.tensor_scalar_add(t1[:, :W * D], t1[:, :W * D], 1.0)
                nc.vector.tensor_sub(out=t1[:, :W * D], in0=t1[:, :W * D],
                                     in1=m2[:, :W * D])
                nc.vector.tensor_sub(out=t1[:, :W * D], in0=t1[:, :W * D],
                                     in1=exg[:, :W * D])
                nc.vector.tensor_reduce(
                    out=klcols[:, w0:w0 + W],
                    in_=t1[:, :W * D].rearrange("p (c d) -> p c d", d=D),
                    axis=AX.X, op=ALU.add)
            nc.vector.tensor_tensor(out=klcols[:], in0=klcols[:],
                                    in1=kmask[:], op=ALU.mult)
            ktot = wp.tile([128, 1], F32, tag="ktot")
            nc.vector.tensor_reduce(out=ktot[:], in_=klcols[:], axis=AX.X,
                                    op=ALU.add)
            part_sum(ktot[:], 1)

            nc.sync.dma_start(out=partials[:], in_=psb[:])

    return nc


# --------------------------------------------------------------------------
# entry
# --------------------------------------------------------------------------

def prepare(inputs, c):
    """Returns (nc, in_maps)."""
    NC = c["NC"]
    per_core, ego, perm_row, uplan, NI = host_prep(inputs, c)
    c["UPLAN"] = uplan
    c["NI"] = NI

    users0 = np.asarray(inputs["users"]).astype(np.int64)
    pos0 = np.asarray(inputs["pos_items"]).astype(np.int64)
    neg0 = np.asarray(inputs["neg_items"]).astype(np.int64)
    N_USERS, B, BSH = c["N_USERS"], c["B"], c["BSH"]

    def cycb(v):
        m = len(v) // 128
        return v.reshape(m, 128).T.astype(np.int32)

    eps_np = np.asarray(inputs["eps"], dtype=np.float32)
    ui_np = np.asarray(inputs["user_intent"], dtype=np.float32)
    ii_np = np.asarray(inputs["item_intent"], dtype=np.float32)
    lw_np = np.asarray(inputs["lin_w"], dtype=np.float32)
    lb_rep = np.tile(np.asarray(inputs["lin_b"],
                                dtype=np.float32)[None, :], (128, 1))

    in_maps = []
    for k in range(NC):
        rot = np.roll(np.arange(B), -k * BSH)
        users, pos, neg = users0[rot], pos0[rot], neg0[rot]
        sh = slice(0, BSH)
        idx_b = np.concatenate([
            cycb(perm_row[users]),                    # OFF_IU (full)
            cycb(perm_row[N_USERS + pos]),            # OFF_IP (full)
            cycb(perm_row[users[sh]]),                # OFF_US
            cycb(perm_row[N_USERS + pos[sh]]),        # OFF_PS
            cycb(perm_row[N_USERS + neg[sh]]),        # OFF_NS
            cycb(users[sh]),                          # OFF_EU
            cycb(N_USERS + pos[sh]),                  # OFF_EP
            cycb(users[sh]),                          # OFF_GU
            cycb(N_USERS + pos[sh]),                  # OFF_GP
            cycb(N_USERS + neg[sh]),                  # OFF_GN
        ], axis=1)
        pk = per_core[k]
        in_maps.append(dict(
            ego_perm=pk["ego_perm"], idx_spmm=pk["idx_spmm"],
            dinv=pk["dinv"], dinv2=pk["dinv2"], kmask=pk["kmask"],
            idx_b=idx_b, eps=eps_np, ego_full=ego, user_intent=ui_np,
            item_intent=ii_np, lin_w=lw_np, lin_b_rep=lb_rep))

    nc = build_bass(c)
    split_multi_waits(nc)
    return nc, in_maps


def combine(results, c):
    NC, B, N = c["NC"], c["B"], c["N"]
    P = np.stack([np.asarray(results[k]["partials"][0], dtype=np.float64)
                  for k in range(NC)])
    bpr = P[:, 0].sum() / B
    kl = c["KL_REG"] * (-0.5 * P[:, 1].sum()) / N
    gen_loss = np.float32(bpr + kl)
    cl_loss = np.float32(c["SSL_REG"] * (-(P[:, 2].sum()) - P[:, 3].sum()) / B)
    emb_loss = np.float32(c["EMB_REG"] * P[:, 4].sum())
    int_loss = np.float32(c["INT_REG"] * P[0, 5])
    return (gen_loss, cl_loss, emb_loss, int_loss)


def kernel(**inputs):
    c = derive(default_cfg())
    nc, in_maps = prepare(inputs, c)
    res = run_bass_kernel_spmd(nc, in_maps, list(range(c["NC"])))
    return combine(res.results, c)



# revision 26
# speedup vs baseline: 1.1239x; 1.0387x over previous
"""DVGCL (GNN message passing + contrastive losses) on 8 Trainium2 cores.

Sharding: node dim N split 8 ways by destination; each shard degree-sorted and
laid out cyclically (pos j -> partition j%128, col j//128); the permutation is
folded into every index array on the host. The symmetric normalization
separates (g = d_inv[h] d_inv[t]), so propagation gathers the pre-scaled table
y = d_inv * cur unweighted and rescales shard outputs. Per 128-dest tile,
indirect_dma_start with compute_op=add accumulates gathered rows in SBUF via
the DMA CCE units, 4 slot columns per op (amortizing the ~1us SWDGE fixed
descriptor-generation cost) into 4 parallel accumulator lanes reduced on DVE;
sentinel indices are skipped via bounds_check. Ops round-robin across a
16-tile window so same-tile CCE-add chains keep slack.
Layers are separated by AllGather of the 8 shard updates (double-buffered
table). Losses are computed as per-core partials (batch rotated per core so
its 1/8 slice is always at columns [0, BSHC)) and combined on the host.

Walrus codegen accepts at most ONE sync wait per instruction, so
split_multi_waits hoists extras onto same-engine NoOps after Tile scheduling.
"""
import math
import numpy as np

import concourse.bass as bass
import concourse.mybir as mybir
import concourse.tile as tile
from concourse.bass_utils import run_bass_kernel_spmd
from concourse.masks import make_identity

F32 = mybir.dt.float32
I32 = mybir.dt.int32
AX = mybir.AxisListType
ALU = mybir.AluOpType
ACTF = mybir.ActivationFunctionType

SENT = 1 << 20


def default_cfg():
    return dict(
        N_USERS=50000, N_ITEMS=100000, D=64, N_LAYERS=3, N_INTENTS=128,
        T_SIZE=32, TEMP=0.2, KL_REG=0.01, EMB_REG=1e-5, INT_REG=1e-5,
        SSL_REG=0.1, B=4096, NC=8,
    )


def derive(cfg):
    c = dict(cfg)
    c["N"] = c["N_USERS"] + c["N_ITEMS"]
    assert c["N"] % c["NC"] == 0
    c["SHARD"] = c["N"] // c["NC"]
    c["PC"] = math.ceil(c["SHARD"] / 128)
    c["SPAD"] = 128 * c["PC"]
    c["TROWS"] = c["NC"] * c["SPAD"]
    assert c["B"] % 128 == 0 and (c["B"] // c["NC"]) % 128 == 0
    c["BCOLS"] = c["B"] // 128
    c["BSH"] = c["B"] // c["NC"]
    c["BSHC"] = c["BSH"] // 128
    return c


# --------------------------------------------------------------------------
# wait splitting post-pass (walrus: max 1 sync wait per instruction)
# --------------------------------------------------------------------------

def split_multi_waits(nc, max_waits=1):
    n = 0
    for f in nc.m.functions:
        for b in f.blocks:
            insts = b.instructions
            items = list(insts)
            out = []
            for i in items:
                si = i.sync_info
                w = list(si.on_wait) if si and si.on_wait else []
                if len(w) > max_waits:
                    for x in w[:-max_waits]:
                        n += 1
                        out.append(mybir.InstNoOp(
                            name=f"waitsplit-{n}",
                            sync_info=mybir.SyncInfo(on_wait=[x], on_update=[]),
                            engine=i.engine, bass_nofuse=True))
                    si.on_wait = w[-max_waits:]
                out.append(i)
            insts.clear()
            insts.extend(out)
    return n


# --------------------------------------------------------------------------
# host prep
# --------------------------------------------------------------------------

def host_prep(inputs, c):
    N, NC, SHARD, SPAD, PC, D = (c["N"], c["NC"], c["SHARD"], c["SPAD"],
                                 c["PC"], c["D"])
    h = np.asarray(inputs["h_list"]).astype(np.int64)
    t = np.asarray(inputs["t_list"]).astype(np.int64)

    deg = np.bincount(h, minlength=N).astype(np.int64)
    with np.errstate(divide="ignore"):
        d_inv = (deg.astype(np.float64) ** -0.5).astype(np.float32)

    perm_row = np.empty(N, dtype=np.int64)
    inv_order = []
    for k in range(NC):
        lo = k * SHARD
        order = np.argsort(deg[lo:lo + SHARD], kind="stable")
        perm_row[lo + order] = k * SPAD + np.arange(SHARD)
        inv_order.append(lo + order)

    dest_pos = perm_row[h]
    eorder = np.argsort(dest_pos, kind="stable")
    dpos_s = dest_pos[eorder]
    src_s = perm_row[t[eorder]]

    ego = np.concatenate([
        np.asarray(inputs["user_emb"], dtype=np.float32),
        np.asarray(inputs["item_emb"], dtype=np.float32),
    ], axis=0)

    # per-core per-tile slot columns
    core_cols = []       # list of dict[(tau, s)] -> int32[128]
    core_smax = []       # per core: [PC] slot counts
    core_zero = []       # per core: [PC] tile has a zero-degree dest
    for k in range(NC):
        base = k * SPAD
        lo_i = np.searchsorted(dpos_s, base)
        hi_i = np.searchsorted(dpos_s, base + SHARD)
        dj = dpos_s[lo_i:hi_i] - base
        sj = src_s[lo_i:hi_i]
        degl = np.zeros(SPAD, dtype=np.int64)
        np.add.at(degl, dj, 1)
        starts = np.zeros(SPAD + 1, dtype=np.int64)
        np.cumsum(degl, out=starts[1:])
        cols = {}
        smax = np.zeros(PC, dtype=np.int64)
        zero = np.zeros(PC, dtype=bool)
        for tau in range(PC):
            jlo = tau * 128
            dtile = degl[jlo:jlo + 128]
            smax[tau] = int(dtile.max())
            zero[tau] = bool((dtile == 0).any())
            for s in range(smax[tau]):
                col = np.full(128, SENT, dtype=np.int64)
                sel = dtile > s
                col[sel] = sj[starts[jlo:jlo + 128][sel] + s]
                cols[(tau, s)] = col.astype(np.int32)
        core_cols.append(cols)
        core_smax.append(smax)
        core_zero.append(zero)

    # SPMD union plan
    smax_u = np.max(np.stack(core_smax), axis=0)
    zero_u = np.any(np.stack(core_zero), axis=0)
    uplan = []
    for tau in range(PC):
        s = int(smax_u[tau])
        if s == 0:
            uplan.append(dict(tau=tau, memset=True, ops=[]))
        elif zero_u[tau]:
            uplan.append(dict(tau=tau, memset=True, ops=["add"] * s))
        else:
            uplan.append(dict(tau=tau, memset=False,
                              ops=["bypass"] + ["add"] * (s - 1)))
    NI = max(1, int(smax_u.sum()))

    per_core = []
    for k in range(NC):
        out_cols = []
        for e in uplan:
            for s in range(len(e["ops"])):
                col = core_cols[k].get((e["tau"], s))
                if col is None:
                    col = np.full(128, SENT, dtype=np.int32)
                out_cols.append(col)
        idx_spmm = (np.stack(out_cols, axis=1) if out_cols
                    else np.zeros((128, 1), np.int32))

        def cyc(vec):
            return vec.reshape(PC, 128).T.copy()

        dloc = np.zeros(SPAD, dtype=np.float32)
        dloc[:SHARD] = d_inv[inv_order[k]]
        mask = np.zeros(SPAD, dtype=np.float32)
        mask[:SHARD] = 1.0
        egp = np.zeros((SPAD, D), dtype=np.float32)
        egp[:SHARD] = ego[inv_order[k]]
        per_core.append(dict(
            idx_spmm=idx_spmm, dinv=cyc(dloc), dinv2=cyc(dloc * dloc),
            kmask=cyc(mask), ego_perm=egp))

    return per_core, ego, perm_row, uplan, NI


# --------------------------------------------------------------------------
# device program
# --------------------------------------------------------------------------

def build_bass(c):
    NC, D, PC, SPAD, TROWS = c["NC"], c["D"], c["PC"], c["SPAD"], c["TROWS"]
    BC, BSHC, NI = c["BCOLS"], c["BSHC"], c["NI"]
    NINT, TS, NL = c["N_INTENTS"], c["T_SIZE"], c["N_LAYERS"]
    TEMP = c["TEMP"]
    uplan = c["UPLAN"]
    NB = 2 * BC + 8 * BSHC

    nc = bass.Bass(num_devices=NC)

    ego_perm = nc.dram_tensor("ego_perm", [SPAD, D], F32, kind="ExternalInput")
    idx_spmm = nc.dram_tensor("idx_spmm", [128, NI], I32, kind="ExternalInput")
    dinv_in = nc.dram_tensor("dinv", [128, PC], F32, kind="ExternalInput")
    dinv2_in = nc.dram_tensor("dinv2", [128, PC], F32, kind="ExternalInput")
    kmask_in = nc.dram_tensor("kmask", [128, PC], F32, kind="ExternalInput")
    idx_b_in = nc.dram_tensor("idx_b", [128, NB], I32, kind="ExternalInput")
    eps_in = nc.dram_tensor("eps", [c["N"], D], F32, kind="ExternalInput")
    ego_full = nc.dram_tensor("ego_full", [c["N"], D], F32,
                              kind="ExternalInput")
    ui_in = nc.dram_tensor("user_intent", [D, NINT], F32, kind="ExternalInput")
    ii_in = nc.dram_tensor("item_intent", [D, NINT], F32, kind="ExternalInput")
    lw_in = nc.dram_tensor("lin_w", [D, TS], F32, kind="ExternalInput")
    lb_in = nc.dram_tensor("lin_b_rep", [128, D], F32, kind="ExternalInput")

    partials = nc.dram_tensor("partials", [1, 16], F32, kind="ExternalOutput")

    BF16 = mybir.dt.bfloat16
    yA = nc.dram_tensor("yA", [TROWS, D], BF16, addr_space="Shared")
    yB = nc.dram_tensor("yB", [TROWS, D], BF16, addr_space="Shared")
    all_emb = nc.dram_tensor("all_emb", [TROWS, D], F32, addr_space="Shared")
    shard_buf = nc.dram_tensor("shard_buf", [SPAD, D], F32)
    shard_bf = nc.dram_tensor("shard_bf", [SPAD, D], BF16)

    groups = [list(range(NC))]

    # batch idx column offsets
    OFF_IU, OFF_IP = 0, BC
    OFF_US = 2 * BC                 # ua shard (perm)
    OFF_PS = OFF_US + BSHC          # ia pos shard (perm)
    OFF_NS = OFF_US + 2 * BSHC      # ia neg shard (perm)
    OFF_EU = OFF_US + 3 * BSHC      # eps users shard (orig)
    OFF_EP = OFF_US + 4 * BSHC      # eps pos shard (orig)
    OFF_GU = OFF_US + 5 * BSHC      # ego users shard (orig)
    OFF_GP = OFF_US + 6 * BSHC
    OFF_GN = OFF_US + 7 * BSHC

    with tile.TileContext(nc) as tc:
        with tc.tile_pool(name="const", bufs=1) as cp, \
             tc.tile_pool(name="work", bufs=2) as wp, \
             tc.tile_pool(name="spmm", bufs=20) as sp, \
             tc.tile_pool(name="curp", bufs=8) as curp, \
             tc.tile_pool(name="psum", bufs=1, space="PSUM") as pp:

            ident = cp.tile([128, 128], F32)
            make_identity(nc, ident[:])
            bc_reg = nc.alloc_register(mybir.EngineType.Pool, "bcreg")
            nc.gpsimd.reg_mov(bc_reg, TROWS - 1)
            ones_col = cp.tile([128, 1], F32)
            nc.vector.memset(ones_col[:], 1.0)

            def load(shape, dt, src, name):
                t_ = cp.tile(shape, dt, tag=name)
                nc.sync.dma_start(out=t_[:], in_=src)
                return t_

            idxs = load([128, NI], I32, idx_spmm[:], "idxs")
            dinv = load([128, PC], F32, dinv_in[:], "dinv")
            dinv2 = load([128, PC], F32, dinv2_in[:], "dinv2")
            kmask = load([128, PC], F32, kmask_in[:], "kmask")
            idxb = load([128, NB], I32, idx_b_in[:], "idxb")
            lb_rep = load([128, D], F32, lb_in[:], "lb")
            ui_sb = load([D, NINT], F32, ui_in[:], "ui")
            ii_sb = load([D, NINT], F32, ii_in[:], "ii")
            lw_sb = load([D, TS], F32, lw_in[:], "lw")

            def transpose128(dst_ap, src_ap, P, Fr):
                # src [P, Fr] -> dst [Fr, P]
                ps = pp.tile([128, 128], F32, tag="tps")
                nc.tensor.transpose(out=ps[:Fr, :P], in_=src_ap,
                                    identity=ident[:P, :P])
                nc.vector.tensor_copy(dst_ap, ps[:Fr, :P])

            uiT = cp.tile([NINT, D], F32)
            transpose128(uiT[:], ui_sb[:], D, NINT)
            iiT = cp.tile([NINT, D], F32)
            transpose128(iiT[:], ii_sb[:], D, NINT)
            lwT = cp.tile([TS, D], F32)
            transpose128(lwT[:], lw_sb[:], D, TS)

            acc = cp.tile([128, PC * D], F32)
            nc.sync.dma_start(
                out=acc[:].rearrange("p (c d) -> p c d", d=D),
                in_=ego_perm[:].rearrange("(c p) d -> p c d", p=128))

            def dcol(tbl, tau):
                return tbl[:, tau:tau + 1].to_broadcast([128, D])

            def sbcol(tau):
                return shard_bf[:].rearrange(
                    "(c p) d -> p c d", p=128)[:, tau, :]

            # y0 = d_inv * ego, per column (cast to bf16 for the y table)
            for tau in range(PC):
                yt = sp.tile([128, D], F32, tag="out_t")
                nc.vector.tensor_tensor(
                    out=yt[:], in0=acc[:, tau * D:(tau + 1) * D],
                    in1=dcol(dinv, tau), op=ALU.mult)
                yb = curp.tile([128, D], BF16, tag="ybf")
                nc.vector.tensor_copy(yb[:], yt[:])
                nc.sync.dma_start(out=sbcol(tau), in_=yb[:])
            nc.gpsimd.collective_compute(
                "AllGather", ALU.bypass, replica_groups=groups,
                ins=[shard_bf[:]], outs=[yA[:]])

            # column index of instr (tau, s) inside idx_spmm
            colof = {}
            _ic = 0
            for e in uplan:
                for s in range(len(e["ops"])):
                    colof[(e["tau"], s)] = _ic
                    _ic += 1

            M = 8   # slots per indirect op (amortizes SWDGE fixed cost)
            W = 8   # tiles per window (bounds live out_t tiles)
            ybufs = [yA, yB]
            for layer in range(NL):
                y_in = ybufs[layer % 2]
                y_out = ybufs[(layer + 1) % 2]
                last = (layer == NL - 1)
                for w0 in range(0, len(uplan), W):
                    win = uplan[w0:w0 + W]
                    outs = {}
                    for e in win:
                        out_t = sp.tile([128, M * D], F32, tag="out_t")
                        outs[e["tau"]] = out_t
                        nc.vector.memset(out_t[:], 0.0)
                    gmax = max((-(-len(e["ops"]) // M) for e in win),
                               default=0)
                    # group-major round-robin across the window so same-tile
                    # CCE-add chains keep ~W ops of slack
                    for g in range(gmax):
                        for e in win:
                            ns = len(e["ops"])
                            if g * M >= ns:
                                continue
                            mw = min(M, ns - g * M)
                            icol = colof[(e["tau"], g * M)]
                            nc.gpsimd.indirect_dma_start(
                                out=outs[e["tau"]][:, :mw * D],
                                out_offset=None,
                                in_=y_in[:],
                                in_offset=bass.IndirectOffsetOnAxis(
                                    ap=idxs[:, icol:icol + mw], axis=0),
                                bounds_check=bc_reg, oob_is_err=False,
                                compute_op=ALU.add)
                    for e in win:
                        tau = e["tau"]
                        out_t = outs[tau]
                        red = curp.tile([128, D], F32, tag="red")
                        nc.vector.tensor_reduce(
                            out=red[:],
                            in_=out_t[:].rearrange("p (m d) -> p d m", d=D),
                            axis=AX.X, op=ALU.add)
                        aslice = acc[:, tau * D:(tau + 1) * D]
                        cur = curp.tile([128, D], F32, tag="cur")
                        nc.vector.tensor_tensor(out=cur[:], in0=red[:],
                                                in1=dcol(dinv, tau),
                                                op=ALU.mult)
                        nc.vector.tensor_add(out=aslice, in0=aslice,
                                             in1=cur[:])
                        if not last:
                            nc.vector.tensor_tensor(
                                out=red[:], in0=red[:], in1=dcol(dinv2, tau),
                                op=ALU.mult)
                            yb = curp.tile([128, D], BF16, tag="ybf")
                            nc.vector.tensor_copy(yb[:], red[:])
                            nc.sync.dma_start(out=sbcol(tau), in_=yb[:])
                if not last:
                    nc.gpsimd.collective_compute(
                        "AllGather", ALU.bypass, replica_groups=groups,
                        ins=[shard_bf[:]], outs=[y_out[:]])

            nc.sync.dma_start(
                out=shard_buf[:].rearrange("(c p) d -> p c d", p=128),
                in_=acc[:].rearrange("p (c d) -> p c d", d=D))
            nc.gpsimd.collective_compute(
                "AllGather", ALU.bypass, replica_groups=groups,
                ins=[shard_buf[:]], outs=[all_emb[:]])

            # ---------------- downstream ----------------

            def gather1(src, col, dst_ap):
                nc.gpsimd.indirect_dma_start(
                    out=dst_ap, out_offset=None, in_=src[:],
                    in_offset=bass.IndirectOffsetOnAxis(
                        ap=idxb[:, col:col + 1], axis=0))

            def gather_set(src, off, ncols, tag, pool=cp):
                tl = pool.tile([128, ncols * D], F32, tag=tag)
                for q in range(ncols):
                    gather1(src, off + q, tl[:, q * D:(q + 1) * D])
                return tl

            ua_sh = gather_set(all_emb, OFF_US, BSHC, "ua_sh")
            iap_sh = gather_set(all_emb, OFF_PS, BSHC, "iap_sh")
            ian_sh = gather_set(all_emb, OFF_NS, BSHC, "ian_sh")
            eps_u = gather_set(eps_in, OFF_EU, BSHC, "eps_u")
            eps_p = gather_set(eps_in, OFF_EP, BSHC, "eps_p")
            ego_u = gather_set(ego_full, OFF_GU, BSHC, "ego_u")
            ego_p = gather_set(ego_full, OFF_GP, BSHC, "ego_p")
            ego_n = gather_set(ego_full, OFF_GN, BSHC, "ego_n")

            def normalize_rows(x_ap, ncols):
                for q in range(ncols):
                    sl = x_ap[:, q * D:(q + 1) * D]
                    sq = wp.tile([128, D], F32, tag="sqj")
                    ss = wp.tile([128, 1], F32, tag="ssj")
                    nc.scalar.activation(sq[:], sl, ACTF.Square,
                                         accum_out=ss[:])
                    ls = wp.tile([128, 1], F32, tag="rsj")
                    nc.scalar.activation(ls[:], ss[:], ACTF.Ln)
                    rn = wp.tile([128, 1], F32, tag="rnj")
                    nc.scalar.activation(rn[:], ls[:], ACTF.Exp, scale=-0.5)
                    nc.vector.tensor_scalar_mul(sl, sl, rn[:])

            def intent_pipe(gsrc_off, w_sb, wT_sb, tag):
                """Full-batch intent; returns (shard normalized [128,BSHC*D],
                e2T [D, B])."""
                sh_n = cp.tile([128, BSHC * D], F32, tag=f"in_{tag}")
                e2T = cp.tile([D, BC * 128], F32, tag=f"iT_{tag}")
                for q in range(BC):
                    tl = wp.tile([128, D], F32, tag="itl")
                    gather1(all_emb, gsrc_off + q, tl[:])
                    tT = wp.tile([D, 128], F32, tag="tT")
                    transpose128(tT[:], tl[:], 128, D)
                    zp = pp.tile([128, NINT], F32, tag="zp")
                    nc.tensor.matmul(out=zp[:], lhsT=tT[:], rhs=w_sb[:])
                    z = wp.tile([128, NINT], F32, tag="z")
                    nc.vector.tensor_copy(z[:], zp[:])
                    mx = wp.tile([128, 1], F32, tag="mx")
                    nc.vector.tensor_reduce(out=mx[:], in_=z[:], axis=AX.X,
                                            op=ALU.max)
                    nmx = wp.tile([128, 1], F32, tag="nmx")
                    nc.scalar.mul(nmx[:], mx[:], -1.0)
                    ex = wp.tile([128, NINT], F32, tag="ex")
                    se = wp.tile([128, 1], F32, tag="se")
                    nc.scalar.activation(ex[:], z[:], ACTF.Exp, bias=nmx[:],
                                         accum_out=se[:])
                    rse = wp.tile([128, 1], F32, tag="rse")
                    nc.vector.reciprocal(rse[:], se[:])
                    nc.vector.tensor_scalar_mul(ex[:], ex[:], rse[:])
                    exT = wp.tile([NINT, 128], F32, tag="exT")
                    transpose128(exT[:], ex[:], 128, NINT)
                    op_ = pp.tile([128, D], F32, tag="op")
                    nc.tensor.matmul(out=op_[:], lhsT=exT[:], rhs=wT_sb[:])
                    onrm = wp.tile([128, D], F32, tag="onrm")
                    nc.vector.tensor_copy(onrm[:], op_[:])
                    normalize_rows(onrm[:], 1)
                    if q < BSHC:
                        nc.vector.tensor_copy(
                            sh_n[:, q * D:(q + 1) * D], onrm[:])
                    transpose128(e2T[:, q * 128:(q + 1) * 128], onrm[:],
                                 128, D)
                return sh_n, e2T

            u_i_n, u_i_T = intent_pipe(OFF_IU, ui_sb, uiT, "u")
            i_i_n, i_i_T = intent_pipe(OFF_IP, ii_sb, iiT, "i")

            def gen_pipe(mean_tl, eps_tl, tag):
                gen_n = cp.tile([128, BSHC * D], F32, tag=f"gen_{tag}")
                genT = cp.tile([D, BSHC * 128], F32, tag=f"genT_{tag}")
                for q in range(BSHC):
                    msl = mean_tl[:, q * D:(q + 1) * D]
                    sp_t = wp.tile([128, TS], F32, tag="sp_t")
                    nc.scalar.activation(sp_t[:], msl[:, :TS], ACTF.Exp)
                    nc.vector.tensor_scalar_add(sp_t[:], sp_t[:], 1.0)
                    nc.scalar.activation(sp_t[:], sp_t[:], ACTF.Ln)
                    spT = wp.tile([TS, 128], F32, tag="spT")
                    transpose128(spT[:], sp_t[:], 128, TS)
                    stp = pp.tile([128, D], F32, tag="stp")
                    nc.tensor.matmul(out=stp[:], lhsT=spT[:], rhs=lwT[:])
                    std = wp.tile([128, D], F32, tag="std")
                    nc.vector.tensor_add(out=std[:], in0=stp[:],
                                         in1=lb_rep[:])
                    nc.vector.tensor_scalar_add(std[:], std[:], 1e-8)
                    g = gen_n[:, q * D:(q + 1) * D]
                    nc.vector.tensor_tensor(
                        out=g, in0=eps_tl[:, q * D:(q + 1) * D], in1=std[:],
                        op=ALU.mult)
                    nc.vector.tensor_add(out=g, in0=g, in1=msl)
                normalize_rows(gen_n[:], BSHC)
                for q in range(BSHC):
                    transpose128(genT[:, q * 128:(q + 1) * 128],
                                 gen_n[:, q * D:(q + 1) * D], 128, D)
                return gen_n, genT

            u_gen_n, u_gen_T = gen_pipe(ua_sh, eps_u, "gu")
            i_gen_n, i_gen_T = gen_pipe(iap_sh, eps_p, "gi")

            psb = cp.tile([1, 16], F32)
            nc.vector.memset(psb[:], 0.0)

            def part_sum(vec_ap, slot, P=128):
                ps = pp.tile([1, 1], F32, tag="pscal")
                nc.tensor.matmul(out=ps[:], lhsT=vec_ap, rhs=ones_col[:P, :])
                nc.vector.tensor_add(out=psb[:, slot:slot + 1],
                                     in0=psb[:, slot:slot + 1], in1=ps[:])

            def infonce(e1_n, e1_T, e2sh_n, e2_T, slot):
                lgs = wp.tile([128, BSHC], F32, tag="lgs")
                for q in range(BSHC):
                    prod = wp.tile([128, D], F32, tag="prod")
                    nc.vector.tensor_tensor(
                        out=prod[:], in0=e1_n[:, q * D:(q + 1) * D],
                        in1=e2sh_n[:, q * D:(q + 1) * D], op=ALU.mult)
                    pdot = wp.tile([128, 1], F32, tag="pdot")
                    nc.vector.tensor_reduce(out=pdot[:], in_=prod[:],
                                            axis=AX.X, op=ALU.add)
                    pex = wp.tile([128, 1], F32, tag="pex")
                    nc.scalar.activation(pex[:], pdot[:], ACTF.Exp,
                                         scale=1.0 / TEMP)
                    nss = wp.tile([128, BC], F32, tag="nss")
                    for ch in range(BC):
                        zp = pp.tile([128, 128], F32, tag="zneg")
                        nc.tensor.matmul(
                            out=zp[:], lhsT=e1_T[:, q * 128:(q + 1) * 128],
                            rhs=e2_T[:, ch * 128:(ch + 1) * 128])
                        ju = wp.tile([128, 128], F32, tag="ju")
                        nc.scalar.activation(
                            ju[:], zp[:], ACTF.Exp, scale=1.0 / TEMP,
                            accum_out=nss[:, ch:ch + 1])
                    nsum = wp.tile([128, 1], F32, tag="nsum")
                    nc.vector.tensor_reduce(out=nsum[:], in_=nss[:],
                                            axis=AX.X, op=ALU.add)
                    nc.vector.tensor_scalar_add(nsum[:], nsum[:], 1e-8)
                    rn = wp.tile([128, 1], F32, tag="rng")
                    nc.vector.reciprocal(rn[:], nsum[:])
                    qv = wp.tile([128, 1], F32, tag="qv")
                    nc.vector.tensor_tensor(out=qv[:], in0=pex[:], in1=rn[:],
                                            op=ALU.mult)
                    nc.vector.tensor_scalar_add(qv[:], qv[:], 1e-8)
                    nc.scalar.activation(lgs[:, q:q + 1], qv[:], ACTF.Ln)
                tot = wp.tile([128, 1], F32, tag="lgt")
                nc.vector.tensor_reduce(out=tot[:], in_=lgs[:], axis=AX.X,
                                        op=ALU.add)
                part_sum(tot[:], slot)

            infonce(u_gen_n, u_gen_T, u_i_n, u_i_T, 2)
            infonce(i_gen_n, i_gen_T, i_i_n, i_i_T, 3)

            # BPR
            dsc = wp.tile([128, BSHC], F32, tag="dsc")
            for q in range(BSHC):
                pr = wp.tile([128, D], F32, tag="bprp")
                nc.vector.tensor_tensor(
                    out=pr[:], in0=ua_sh[:, q * D:(q + 1) * D],
                    in1=iap_sh[:, q * D:(q + 1) * D], op=ALU.mult)
                ps_ = wp.tile([128, 1], F32, tag="bps")
                nc.vector.tensor_reduce(out=ps_[:], in_=pr[:], axis=AX.X,
                                        op=ALU.add)
                nr = wp.tile([128, D], F32, tag="bprn")
                nc.vector.tensor_tensor(
                    out=nr[:], in0=ua_sh[:, q * D:(q + 1) * D],
                    in1=ian_sh[:, q * D:(q + 1) * D], op=ALU.mult)
                ns_ = wp.tile([128, 1], F32, tag="bns")
                nc.vector.tensor_reduce(out=ns_[:], in_=nr[:], axis=AX.X,
                                        op=ALU.add)
                nc.vector.tensor_sub(out=dsc[:, q:q + 1], in0=ns_[:],
                                     in1=ps_[:])
            spl = wp.tile([128, BSHC], F32, tag="spl")
            nc.scalar.activation(spl[:], dsc[:], ACTF.Exp)
            nc.vector.tensor_scalar_add(spl[:], spl[:], 1.0)
            nc.scalar.activation(spl[:], spl[:], ACTF.Ln)
            bps = wp.tile([128, 1], F32, tag="bpst")
            nc.vector.tensor_reduce(out=bps[:], in_=spl[:], axis=AX.X,
                                    op=ALU.add)
            part_sum(bps[:], 0)

            # emb partial
            esums = wp.tile([128, 4], F32, tag="esums")
            for j, tl in enumerate([ego_u, ego_p, ego_n]):
                jk = wp.tile([128, BSHC * D], F32, tag="jnk")
                nc.scalar.activation(jk[:], tl[:], ACTF.Square,
                                     accum_out=esums[:, j:j + 1])
            nc.vector.memset(esums[:, 3:4], 0.0)
            etot = wp.tile([128, 1], F32, tag="etot")
            nc.vector.tensor_reduce(out=etot[:], in_=esums[:], axis=AX.X,
                                    op=ALU.add)
            part_sum(etot[:], 4)

            # int partial
            isums = wp.tile([D, 2], F32, tag="isums")
            for j, tl in enumerate([ui_sb, ii_sb]):
                jk2 = wp.tile([D, NINT], F32, tag="jnk2")
                nc.scalar.activation(jk2[:], tl[:], ACTF.Square,
                                     accum_out=isums[:, j:j + 1])
            itot = wp.tile([D, 1], F32, tag="itot")
            nc.vector.tensor_reduce(out=itot[:], in_=isums[:], axis=AX.X,
                                    op=ALU.add)
            part_sum(itot[:], 5, P=D)

            # KL over own shard (from acc)
            KW = 8
            klcols = cp.tile([128, PC], F32)
            for g in range(math.ceil(PC / KW)):
                w0 = g * KW
                W = min(KW, PC - w0)
                mean_g = acc[:, w0 * D:(w0 + W) * D]
                spg = wp.tile([128, KW * TS], F32, tag="spg")
                nc.scalar.activation(
                    spg[:, :W * TS].rearrange("p (c d) -> p c d", d=TS),
                    acc[:, w0 * D:].rearrange(
                        "p (c d) -> p c d", d=D)[:, 0:W, 0:TS],
                    ACTF.Exp)
                nc.vector.tensor_scalar_add(spg[:, :W * TS], spg[:, :W * TS],
                                            1.0)
                nc.scalar.activation(spg[:, :W * TS], spg[:, :W * TS],
                                     ACTF.Ln)
                stdg = wp.tile([128, KW * D], F32, tag="stdg")
                for w in range(W):
                    spT = wp.tile([TS, 128], F32, tag="spTk")
                    transpose128(spT[:], spg[:, w * TS:(w + 1) * TS], 128, TS)
                    stp = pp.tile([128, D], F32, tag="stp")
                    nc.tensor.matmul(out=stp[:], lhsT=spT[:], rhs=lwT[:])
                    sw = stdg[:, w * D:(w + 1) * D]
                    nc.vector.tensor_add(out=sw, in0=stp[:], in1=lb_rep[:])
                    nc.vector.tensor_scalar_add(sw, sw, 1e-8)
                m2 = wp.tile([128, KW * D], F32, tag="m2")
                nc.scalar.activation(m2[:, :W * D], mean_g, ACTF.Square)
                exg = wp.tile([128, KW * D], F32, tag="exg")
                nc.scalar.activation(exg[:, :W * D], stdg[:, :W * D],
                                     ACTF.Exp, scale=2.0)
                t1 = wp.tile([128, KW * D], F32, tag="t1")
                nc.scalar.activation(t1[:, :W * D], stdg[:, :W * D],
                                     ACTF.Copy, bias=0.0, scale=2.0)
                nc.vector.tensor_scalar_add(t1[:, :W * D], t1[:, :W * D], 1.0)
                nc.vector.tensor_sub(out=t1[:, :W * D], in0=t1[:, :W * D],
                                     in1=m2[:, :W * D])
                nc.vector.tensor_sub(out=t1[:, :W * D], in0=t1[:, :W * D],
                                     in1=exg[:, :W * D])
                nc.vector.tensor_reduce(
                    out=klcols[:, w0:w0 + W],
                    in_=t1[:, :W * D].rearrange("p (c d) -> p c d", d=D),
                    axis=AX.X, op=ALU.add)
            nc.vector.tensor_tensor(out=klcols[:], in0=klcols[:],
                                    in1=kmask[:], op=ALU.mult)
            ktot = wp.tile([128, 1], F32, tag="ktot")
            nc.vector.tensor_reduce(out=ktot[:], in_=klcols[:], axis=AX.X,
                                    op=ALU.add)
            part_sum(ktot[:], 1)

            nc.sync.dma_start(out=partials[:], in_=psb[:])

    return nc


# --------------------------------------------------------------------------
# entry
# --------------------------------------------------------------------------

def prepare(inputs, c):
    """Returns (nc, in_maps)."""
    NC = c["NC"]
    per_core, ego, perm_row, uplan, NI = host_prep(inputs, c)
    c["UPLAN"] = uplan
    c["NI"] = NI

    users0 = np.asarray(inputs["users"]).astype(np.int64)
    pos0 = np.asarray(inputs["pos_items"]).astype(np.int64)
    neg0 = np.asarray(inputs["neg_items"]).astype(np.int64)
    N_USERS, B, BSH = c["N_USERS"], c["B"], c["BSH"]

    def cycb(v):
        m = len(v) // 128
        return v.reshape(m, 128).T.astype(np.int32)

    eps_np = np.asarray(inputs["eps"], dtype=np.float32)
    ui_np = np.asarray(inputs["user_intent"], dtype=np.float32)
    ii_np = np.asarray(inputs["item_intent"], dtype=np.float32)
    lw_np = np.asarray(inputs["lin_w"], dtype=np.float32)
    lb_rep = np.tile(np.asarray(inputs["lin_b"],
                                dtype=np.float32)[None, :], (128, 1))

    in_maps = []
    for k in range(NC):
        rot = np.roll(np.arange(B), -k * BSH)
        users, pos, neg = users0[rot], pos0[rot], neg0[rot]
        sh = slice(0, BSH)
        idx_b = np.concatenate([
            cycb(perm_row[users]),                    # OFF_IU (full)
            cycb(perm_row[N_USERS + pos]),            # OFF_IP (full)
            cycb(perm_row[users[sh]]),                # OFF_US
            cycb(perm_row[N_USERS + pos[sh]]),        # OFF_PS
            cycb(perm_row[N_USERS + neg[sh]]),        # OFF_NS
            cycb(users[sh]),                          # OFF_EU
            cycb(N_USERS + pos[sh]),                  # OFF_EP
            cycb(users[sh]),                          # OFF_GU
            cycb(N_USERS + pos[sh]),                  # OFF_GP
            cycb(N_USERS + neg[sh]),                  # OFF_GN
        ], axis=1)
        pk = per_core[k]
        in_maps.append(dict(
            ego_perm=pk["ego_perm"], idx_spmm=pk["idx_spmm"],
            dinv=pk["dinv"], dinv2=pk["dinv2"], kmask=pk["kmask"],
            idx_b=idx_b, eps=eps_np, ego_full=ego, user_intent=ui_np,
            item_intent=ii_np, lin_w=lw_np, lin_b_rep=lb_rep))

    nc = build_bass(c)
    split_multi_waits(nc)
    return nc, in_maps


def combine(results, c):
    NC, B, N = c["NC"], c["B"], c["N"]
    P = np.stack([np.asarray(results[k]["partials"][0], dtype=np.float64)
                  for k in range(NC)])
    bpr = P[:, 0].sum() / B
    kl = c["KL_REG"] * (-0.5 * P[:, 1].sum()) / N
    gen_loss = np.float32(bpr + kl)
    cl_loss = np.float32(c["SSL_REG"] * (-(P[:, 2].sum()) - P[:, 3].sum()) / B)
    emb_loss = np.float32(c["EMB_REG"] * P[:, 4].sum())
    int_loss = np.float32(c["INT_REG"] * P[0, 5])
    return (gen_loss, cl_loss, emb_loss, int_loss)


def kernel(**inputs):
    c = derive(default_cfg())
    nc, in_maps = prepare(inputs, c)
    res = run_bass_kernel_spmd(nc, in_maps, list(range(c["NC"])))
    return combine(res.results, c)

